# revision 15
# baseline (speedup 1.0000x reference)
"""Trainium2 Bass kernel for nn_ObjectWordGAT (8-core data parallel).

Self-contained: accepts FULL inputs, shards batch across 8 NeuronCores,
returns the FULL [256, 512] fp32 output.

Warm-path design (the wall clock is dominated by the ~73MB/s axon link and
~0.1s RPC latencies, not HW exec):
  - embeddings are uploaded as per-row int8 (natural row-major layout), and
    dequantized + transposed on-chip (scalar engine affine + PE transposes),
    eliminating both the host-side transpose and half the upload bytes;
  - attention scores s = x @ (W @ att) are computed exactly on host (tiny
    sgemm) and uploaded pre-packed (~0.6MB), removing the quantization error
    from the logit path;
  - wm (head-mean projection) is derived on-chip from wh;
  - the jitted shard_map dispatch closure is built once and cached, so warm
    calls skip retracing/recompiling;
  - all device_puts are issued asynchronously and overlap each other.
"""
import numpy as np
import concourse.mybir as mybir


# ======== gat_core.py ========

from contextlib import ExitStack

from concourse.masks import make_identity

F16 = mybir.dt.float16
F32 = mybir.dt.float32
I8 = mybir.dt.int8
AF = mybir.ActivationFunctionType
ALU = mybir.AluOpType
AX = mybir.AxisListType

D = 512
H = 2
E = 512
No = 36
Nw = 256
NEG = 0.2
NWCHUNK = 8  # word rows are uploaded in this many pipelined chunks


def build_gat(tc, out_ap, ins, nb=32, nblk=4, has_bias=False, dbg=None):
    def tap(name, ap):
        if dbg is not None and name in dbg:
            tc.nc.sync.dma_start(dbg[name][:], ap)

    nc = tc.nc
    xoq, xosc = ins["xoq"], ins["xosc"]
    wh, swp, sA2 = ins["wh"], ins["swp"], ins["sA2"]
    RW, RO = nb * Nw, nb * 64  # obj rows padded to 64 per b
    ROP = nb * No              # packed obj rows (36 per b)
    nbl = nb // nblk
    not_ = nb * Nw // 128      # word row tiles per core
    ntc = not_ // NWCHUNK      # word row tiles per upload chunk
    nto = ROP // 128           # obj row tiles per core (1152/128 = 9)
    assert ROP % 128 == 0
    assert nb % nblk == 0 and nblk % 2 == 0

    ctx = ExitStack()
    with ctx:
        const = ctx.enter_context(tc.tile_pool(name="const", bufs=1))
        # ---- constants ----
        wh_sb = [const.tile([128, 1024], F16, name=f"wh{c}", tag=f"wh{c}") for c in range(4)]
        wm_sb = [const.tile([128, 512], F16, name=f"wm{c}", tag=f"wm{c}") for c in range(4)]
        for c in range(4):
            sl = slice(c * 128, (c + 1) * 128)
            nc.sync.dma_start(wh_sb[c][:], wh[sl, :])
        ident16 = const.tile([128, 128], F16, name="id16", tag="id16")
        ident32 = const.tile([128, 128], F32, name="id32", tag="id32")
        make_identity(nc, ident16[:])
        make_identity(nc, ident32[:])
        ones16 = const.tile([1, 128], F16, name="ones16", tag="ones16")
        nc.vector.memset(ones16[:], 1.0)
        # wm = 0.5 * (wh_head0 + wh_head1), on-chip
        for c in range(4):
            nc.vector.tensor_add(wm_sb[c][:], wh_sb[c][:, 0:512], wh_sb[c][:, 512:1024])
            nc.scalar.mul(wm_sb[c][:], wm_sb[c][:], 0.5)
        # packed attention-score constants (computed on host, exact)
        swp_sb = const.tile([128, nbl * nblk * 8], F16, name="swp", tag="swp")
        nc.sync.dma_start(swp_sb[:], swp[:, :])
        sA2_sb = const.tile([1, nb * 148], F16, name="sA2", tag="sA2")
        nc.sync.dma_start(sA2_sb[:], sA2[:, :])
        # quant scales
        xwsc_sb = [const.tile([128, ntc], F32, name=f"xwsc{i}", tag=f"xwsc{i}")
                   for i in range(NWCHUNK)]
        for i in range(NWCHUNK):
            nc.sync.dma_start(xwsc_sb[i][:], ins[f"xwsc{i}"][:, :])
        xosc_sb = const.tile([128, nto], F32, name="xosc", tag="xosc")
        nc.sync.dma_start(xosc_sb[:], xosc[:, :])
        if has_bias:
            bias_sb = const.tile([128, 4], F32, name="bias128", tag="bias128")
            nc.sync.dma_start(bias_sb[:], ins["bias128"][:, :])
            biasrow_sb = const.tile([1, 512], F16, name="biasrow", tag="biasrow")
            nc.sync.dma_start(biasrow_sb[:], ins["biasrow"][:, :])

        # resident results
        ngrp2 = nb // 2  # obj rows padded: 2 b per 128-row tile
        xto_sb = [const.tile([128, RO], F16, name=f"xto{c}", tag=f"xto{c}") for c in range(4)]
        hobj_sb = const.tile([128, ngrp2 * 1024], F16, name="hobj", tag="hobj")
        uoT_sb = const.tile([128, 4 * RO], F16, name="uoT", tag="uoT")
        outT_sb = const.tile([128, nb * 4], F32, name="outT", tag="outT")

        # ================= PHASE 0: objects dequant + transpose =================
        # xoq [ROP, 512] int8 natural -> xto_sb[c] [128, RO] f16 (64-padded per b)
        with (
            tc.tile_pool(name="sb0", bufs=2) as sb0,
            tc.tile_pool(name="ps0", bufs=2, space="PSUM") as ps0,
        ):
            xtoP_sb = [const.tile([128, ROP], F16, name=f"xtoP{c}", tag=f"xtoP{c}")
                       for c in range(4)]
            for t in range(nto):
                xq_t = sb0.tile([128, 512], I8, name="xq", tag="xq")
                nc.sync.dma_start(xq_t[:], xoq[t * 128:(t + 1) * 128, :])
                xf_t = sb0.tile([128, 512], F16, name="xf", tag="xf")
                nc.scalar.mul(xf_t[:], xq_t[:], xosc_sb[:, t:t + 1])
                pt = ps0.tile([128, 512], F16, name="pt", tag="pt")
                for c in range(4):
                    nc.tensor.transpose(pt[:, c * 128:(c + 1) * 128],
                                        xf_t[:, c * 128:(c + 1) * 128], ident16[:])
                for c in range(4):
                    eng = nc.scalar.copy if (t + c) % 2 == 0 else nc.vector.tensor_copy
                    eng(xtoP_sb[c][:, t * 128:(t + 1) * 128],
                        pt[:, c * 128:(c + 1) * 128])
            # pad 36 -> 64 per b
            for c in range(4):
                nc.gpsimd.memset(
                    xto_sb[c][:].rearrange("p (b n) -> p b n", n=64)[:, :, No:64], 0.0)
                nc.vector.tensor_copy(
                    xto_sb[c][:].rearrange("p (b n) -> p b n", n=64)[:, :, 0:No],
                    xtoP_sb[c][:].rearrange("p (b n) -> p b n", n=No))
        tap("xto0", xto_sb[0][:])

        # ================= PHASE A: objects =================
        with tc.tile_pool(name="psA", bufs=2, space="PSUM") as psA:
            for g in range(ngrp2):
                pt = psA.tile([128, 1024], F32, name="phobj", tag="phobj")
                for he in range(2):
                    for c in range(4):
                        nc.tensor.matmul(
                            pt[:, he * 512:(he + 1) * 512],
                            lhsT=xto_sb[c][:, 128 * g:128 * (g + 1)],
                            rhs=wh_sb[c][:, he * 512:(he + 1) * 512],
                            start=(c == 0), stop=(c == 3),
                        )
                eng = nc.scalar.copy if g % 2 == 0 else nc.vector.tensor_copy
                eng(hobj_sb[:, g * 1024:(g + 1) * 1024], pt[:, :])

        with tc.tile_pool(name="psB", bufs=2, space="PSUM") as psB:
            # upd_obj^T = Wm.T @ Xo^T (+bias on evac)
            nchunks = [(i, min(512, RO - i)) for i in range(0, RO, 512)]
            for ec in range(4):
                for n0, nn in nchunks:
                    pt = psB.tile([128, 512], F32, name="puoT", tag="puoT")
                    for c in range(4):
                        nc.tensor.matmul(
                            pt[:, 0:nn],
                            lhsT=wm_sb[c][:, ec * 128:(ec + 1) * 128],
                            rhs=xto_sb[c][:, n0:n0 + nn],
                            start=(c == 0), stop=(c == 3),
                        )
                    dst = uoT_sb[:, ec * RO + n0: ec * RO + n0 + nn]
                    if has_bias:
                        nc.scalar.activation(dst, pt[:, 0:nn], AF.Identity,
                                             bias=bias_sb[:, ec:ec + 1])
                    elif (ec * len(nchunks) + n0 // 512) % 2 == 0:
                        nc.scalar.copy(dst, pt[:, 0:nn])
                    else:
                        nc.vector.tensor_copy(dst, pt[:, 0:nn])

        tap("hobj", hobj_sb[:])
        tap("uoT", uoT_sb[:])
        tap("sA2", sA2_sb[:])

        # ================= PHASE B: word blocks =================
        with (
            tc.tile_pool(name="sbB", bufs=2) as sbB,
            tc.tile_pool(name="ps_hw", bufs=2, space="PSUM") as ps_hw,
            tc.tile_pool(name="ps_mid", bufs=2, space="PSUM") as ps_mid,
            tc.tile_pool(name="ps_sm", bufs=2, space="PSUM") as ps_sm,
            tc.tile_pool(name="ps_aT", bufs=1, space="PSUM") as ps_aT,
        ):
            for blk in range(nbl):
                gw0 = blk * nblk * Nw  # first word row of block
                nwt = nblk * 2  # 128-row word tiles in block
                ng = nblk * 4   # (bi, whi, h) groups in block
                # ---- dequant + transpose words of this block ----
                xtw_sb = [sbB.tile([128, nblk * 256], F16, name=f"xtw{c}", tag=f"xtw{c}")
                          for c in range(4)]
                for wt8 in range(nwt):
                    t = blk * nwt + wt8
                    ch, tci = t // ntc, t % ntc
                    xq_t = sbB.tile([128, 512], I8, name="xqw", tag="xqw")
                    nc.sync.dma_start(
                        xq_t[:], ins[f"xwq{ch}"][tci * 128:(tci + 1) * 128, :])
                    xf_t = sbB.tile([128, 512], F16, name="xfw", tag="xfw")
                    nc.scalar.mul(xf_t[:], xq_t[:], xwsc_sb[ch][:, tci:tci + 1])
                    pt = ps_aT.tile([128, 512], F16, name="paT", tag="paT")
                    for c in range(4):
                        nc.tensor.transpose(pt[:, c * 128:(c + 1) * 128],
                                            xf_t[:, c * 128:(c + 1) * 128], ident16[:])
                    for c in range(4):
                        eng = nc.scalar.copy if (wt8 + c) % 2 == 0 else nc.vector.tensor_copy
                        eng(xtw_sb[c][:, wt8 * 128:(wt8 + 1) * 128],
                            pt[:, c * 128:(c + 1) * 128])

                # ---- s_word: packed slice of the host-computed scores ----
                sw_sb = swp_sb[:, blk * nwt * 4:(blk + 1) * nwt * 4]
                if blk == 0:
                    tap("sw", sw_sb)

                # ---- spread [128, nblk*148] = s_dst col per (bi,whi,h) ----
                spread_sb = sbB.tile([128, nblk * 148], F16, name="spread", tag="spread")
                src = sw_sb.rearrange("p (b whi f) -> p b whi f",
                                      b=nblk, whi=2)[:, :, :, 2:4]
                src = src.broadcast_to([128, nblk, 2, 2, 37])
                dst = spread_sb[:].rearrange("p (b whi h n) -> p b whi h n",
                                             b=nblk, whi=2, h=2)
                nc.vector.tensor_copy(dst, src)
                # self column (n=36): s_src + s_dst
                swg = sw_sb.rearrange("p (b whi f) -> p b whi f",
                                      b=nblk, whi=2)
                nc.vector.tensor_add(
                    dst[:, :, :, :, 36:37].rearrange("p b whi h n -> p b whi (h n)"),
                    dst[:, :, :, :, 36:37].rearrange("p b whi h n -> p b whi (h n)"),
                    swg[:, :, :, 0:2])

                # ---- L psums + lrelu + exp ----
                L2_sb = sbB.tile([128, nblk * 148], F32, name="L2", tag="L2")
                half = nblk * 148 // 2
                for hf in range(2):
                    p_L = ps_sm.tile([128, half], F32, name="sm", tag="sm")
                    nc.tensor.matmul(
                        p_L[:], lhsT=ones16[:],
                        rhs=sA2_sb[0:1, blk * nblk * 148 + hf * half:][:, 0:half],
                        start=True, stop=False)
                    nc.tensor.matmul(
                        p_L[:], lhsT=ident16[:],
                        rhs=spread_sb[:, hf * half:(hf + 1) * half],
                        start=False, stop=True)
                    ltmp = sbB.tile([128, half], F16, name="ltmp", tag="ltmp")
                    nc.scalar.mul(ltmp[:], p_L[:], NEG)
                    nc.vector.tensor_max(
                        L2_sb[:, hf * half:(hf + 1) * half], p_L[:], ltmp[:])
                expL_sb = sbB.tile([128, nblk * 148], F32, name="expL", tag="expL")
                nc.scalar.activation(expL_sb[:], L2_sb[:], AF.Exp)
                if blk == 0:
                    tap("L2", L2_sb[:])
                    tap("expL", expL_sb[:])

                # ---- den, r, alpha, c ----
                expg = expL_sb[:].rearrange("p (g n) -> p g n", n=37)
                den_sb = sbB.tile([128, ng], F32, name="den", tag="den")
                nc.vector.tensor_reduce(den_sb[:], expg, axis=AX.X, op=ALU.add)
                r_sb = sbB.tile([128, ng], F32, name="r", tag="r")
                nc.vector.reciprocal(r_sb[:], den_sb[:])
                nc.vector.tensor_scalar_mul(r_sb[:], r_sb[:], 0.5)
                alpha_sb = sbB.tile([128, ng * 64], F16, name="alpha", tag="alpha")
                nc.gpsimd.memset(
                    alpha_sb[:].rearrange("p (g n) -> p g n", n=64)[:, :, 36:64],
                    0.0)
                rbc = r_sb[:].broadcast_to([128, ng, 36])
                nc.vector.tensor_mul(
                    alpha_sb[:].rearrange("p (g n) -> p g n", n=64)[:, :, 0:36],
                    expg[:, :, 0:36], rbc)
                c_sb = sbB.tile([128, ng], F32, name="c", tag="c")
                nc.vector.tensor_mul(
                    c_sb[:],
                    expg[:, :, 36:37].rearrange("p g n -> p (g n)"), r_sb[:])
                if blk == 0:
                    tap("den", den_sb[:])
                    tap("alpha", alpha_sb[:])
                    tap("c", c_sb[:])

                # ---- alpha transposes -> aT [128, (nblk/2)*512] ----
                # partition half = b parity; col = pair*512 + h*256 + whi*128
                aT_sb = sbB.tile([128, (nblk // 2) * 512], F16, name="aT",
                                 tag="aT")
                for pr in range(nblk // 2):
                    p_aTt = ps_aT.tile([128, 512], F16, name="paT", tag="paT")
                    for pb in range(2):
                        bi = pr * 2 + pb
                        for whi in range(2):
                            for h in range(2):
                                g = (bi * 2 + whi) * 2 + h
                                nc.tensor.transpose(
                                    p_aTt[64 * pb:64 * pb + 64,
                                          h * 256 + whi * 128:][:, 0:128],
                                    alpha_sb[:, g * 64:(g + 1) * 64],
                                    ident16[:],
                                    tile_position=(0, 64 * pb),
                                )
                    nc.vector.tensor_copy(aT_sb[:, pr * 512:(pr + 1) * 512],
                                          p_aTt[:])

                if blk == 0:
                    tap("aT", aT_sb[:])
                # ---- h_word proj + t + msg + uw per (bi, whi) ----
                t_sb = sbB.tile([128, nwt * 512], F16, name="t", tag="t")
                uw_sb = sbB.tile([128, nwt * 512], F16, name="uw", tag="uw")
                for bi in range(nblk):
                    b = blk * nblk + bi
                    for whi in range(2):
                        wt = bi * 2 + whi
                        g = wt * 2  # (bi, whi, h=0)
                        p_he0 = ps_hw.tile([128, 512], F32, name="hw", tag="hw")
                        for c in range(4):
                            nc.tensor.matmul(
                                p_he0[:],
                                lhsT=xtw_sb[c][:, wt * 128:(wt + 1) * 128],
                                rhs=wh_sb[c][:, 0:512],
                                start=(c == 0), stop=(c == 3))
                        t0_sb = sbB.tile([128, 512], F16, name="t0", tag="t0")
                        nc.scalar.mul(t0_sb[:], p_he0[:], c_sb[:, g:g + 1])
                        p_he1 = ps_hw.tile([128, 512], F32, name="hw", tag="hw")
                        for c in range(4):
                            nc.tensor.matmul(
                                p_he1[:],
                                lhsT=xtw_sb[c][:, wt * 128:(wt + 1) * 128],
                                rhs=wh_sb[c][:, 512:1024],
                                start=(c == 0), stop=(c == 3))
                        t1_sb = sbB.tile([128, 512], F16, name="t1", tag="t1")
                        nc.vector.tensor_scalar_mul(t1_sb[:], p_he1[:],
                                                    c_sb[:, g + 1:g + 2])
                        nc.gpsimd.tensor_add(t_sb[:, wt * 512:(wt + 1) * 512],
                                             t0_sb[:], t1_sb[:])

                        # msg: two K=36 matmuls at row base 64*(b%2)
                        p_msg = ps_mid.tile([128, 512], F32, name="mid", tag="mid")
                        gq, go = b // 2, 64 * (b % 2)
                        acol = (bi // 2) * 512 + whi * 128
                        nc.tensor.matmul(
                            p_msg[:],
                            lhsT=aT_sb[go:go + 36, acol:acol + 128],
                            rhs=hobj_sb[go:go + 36, gq * 1024:gq * 1024 + 512],
                            start=True, stop=False,
                            tile_position=(go, 0))
                        nc.tensor.matmul(
                            p_msg[:],
                            lhsT=aT_sb[go:go + 36, acol + 256:acol + 256 + 128],
                            rhs=hobj_sb[go:go + 36,
                                        gq * 1024 + 512:gq * 1024 + 1024],
                            start=False, stop=not has_bias,
                            tile_position=(go, 0))
                        if has_bias:
                            nc.tensor.matmul(p_msg[:], lhsT=ones16[:],
                                             rhs=biasrow_sb[:],
                                             start=False, stop=True)
                        nc.vector.tensor_add(
                            uw_sb[:, wt * 512:(wt + 1) * 512], p_msg[:],
                            t_sb[:, wt * 512:(wt + 1) * 512])

                if blk == 0:
                    tap("t", t_sb[:])
                    tap("uw", uw_sb[:])
                # ---- uw transposes -> uwT [128, nblk*4*256] ----
                uwT_sb = sbB.tile([128, nblk * 4 * 256], F16, name="uwT", tag="uwT")
                for bi in range(nblk):
                    for ec in range(4):
                        p_uwT = ps_mid.tile([128, 256], F16, name="mid", tag="mid")
                        for whi in range(2):
                            nc.tensor.transpose(
                                p_uwT[:, whi * 128:(whi + 1) * 128],
                                uw_sb[:, (bi * 2 + whi) * 512 + ec * 128:][:, 0:128],
                                ident16[:])
                        dst = uwT_sb[:, (bi * 4 + ec) * 256:
                                     (bi * 4 + ec + 1) * 256]
                        if ec % 2 == 0:
                            nc.scalar.copy(dst, p_uwT[:])
                        else:
                            nc.vector.tensor_copy(dst, p_uwT[:])

                if blk == 0:
                    tap("uwT", uwT_sb[:])
                # ---- C + softmax + attnT ----
                p_attnT = ps_aT.tile([128, nblk * 2 * 36], F16, name="pattnT", tag="pattnT")
                for pair in range(nblk // 2):
                    p_C = ps_sm.tile([128, 256], F32, name="sm", tag="sm")
                    for pb in range(2):
                        bi = pair * 2 + pb
                        b = blk * nblk + bi
                        for ec in range(4):
                            nc.tensor.matmul(
                                p_C[64 * pb:64 * pb + 36, :],
                                lhsT=uoT_sb[:, ec * RO + b * 64:
                                            ec * RO + b * 64 + 36],
                                rhs=uwT_sb[:, (bi * 4 + ec) * 256:
                                           (bi * 4 + ec + 1) * 256],
                                start=(ec == 0), stop=(ec == 3),
                                tile_position=(0, 64 * pb))
                    negmax = sbB.tile([128, 1], F32, name="negmax", tag="negmax")
                    expC = sbB.tile([128, 256], F16, name="expC", tag="expC")
                    den2 = sbB.tile([128, 1], F32, name="den2", tag="den2")
                    rden = sbB.tile([128, 1], F32, name="rden", tag="rden")
                    attn = sbB.tile([128, 256], F16, name="attn", tag="attn")
                    for pb in range(2):
                        rs = slice(64 * pb, 64 * pb + 36)
                        nc.vector.tensor_reduce(negmax[rs], p_C[rs, :], axis=AX.X,
                                                op=ALU.max, negate=True)
                        nc.scalar.activation(expC[rs, :], p_C[rs, :], AF.Exp,
                                             bias=negmax[rs], accum_out=den2[rs])
                        nc.vector.reciprocal(rden[rs], den2[rs])
                        nc.vector.tensor_scalar_mul(rden[rs], rden[rs],
                                                    1.0 / 36.0)
                        nc.vector.tensor_scalar_mul(attn[rs, :], expC[rs, :],
                                                    rden[rs])
                    if blk == 0 and pair == 0:
                        tap("attn", attn[:])
                        tap("expC", expC[:])
                    for pb in range(2):
                        bi = pair * 2 + pb
                        for whi in range(2):
                            nc.tensor.transpose(
                                p_attnT[:, (bi * 2 + whi) * 36:
                                        (bi * 2 + whi + 1) * 36],
                                attn[64 * pb:64 * pb + 36,
                                     whi * 128:(whi + 1) * 128],
                                ident16[64 * pb:64 * pb + 36,
                                        64 * pb:64 * pb + 36],
                                tile_position=(64 * pb, 0))
                attnT_sb = sbB.tile([128, nblk * 2 * 36], F16, name="attnT", tag="attnT")
                nc.vector.tensor_copy(attnT_sb[:], p_attnT[:])
                if blk == 0:
                    tap("attnT", attnT_sb[:])

                # ---- weighted^T + final reduce ----
                for bi in range(nblk):
                    b = blk * nblk + bi
                    p_w = ps_sm.tile([128, 144], F32, name="sm", tag="sm")
                    for ec in range(4):
                        for whi in range(2):
                            nc.tensor.matmul(
                                p_w[:, ec * 36:(ec + 1) * 36],
                                lhsT=uw_sb[:, (bi * 2 + whi) * 512 +
                                           ec * 128:][:, 0:128],
                                rhs=attnT_sb[:, (bi * 2 + whi) * 36:
                                             (bi * 2 + whi + 1) * 36],
                                start=(whi == 0), stop=(whi == 1))
                    nc.vector.tensor_reduce(
                        outT_sb[:, b * 4:(b + 1) * 4],
                        p_w[:].rearrange("p (ec n) -> p ec n", n=36),
                        axis=AX.X, op=ALU.add)

        tap("outT", outT_sb[:])
        # ================= PHASE C: final transpose + store =================
        with tc.tile_pool(name="psC", bufs=1, space="PSUM") as psC:
            assert nb <= 128
            p_out = psC.tile([nb, 512], F32, name="p_out", tag="p_out")
            for ec in range(4):
                src = outT_sb[:].rearrange("p (b ec) -> p ec b", ec=4)[:, ec, :]
                nc.tensor.transpose(p_out[0:nb, ec * 128:(ec + 1) * 128],
                                    src, ident32[:])
            out_sb = const.tile([nb, 512], F32, name="out_sb", tag="out_sb")
            nc.vector.tensor_copy(out_sb[0:nb, :], p_out[0:nb, :])
            nc.sync.dma_start(out_ap[:, :], out_sb[0:nb, :])


# ======== runner.py ========

NCORES = 8
_B_TOTAL = 256
_NB = _B_TOTAL // NCORES  # 32
_NBLK = 4

_built = {}


def _build(nb, nblk, has_bias):
    key = (nb, nblk, has_bias)
    if key in _built:
        return _built[key]
    import concourse.bacc as bacc
    import concourse.tile as tile

    nc = bacc.Bacc(trn_type="TRN2", target_bir_lowering=False, debug=False,
                   num_devices=NCORES)
    f16 = mybir.dt.float16
    f32 = mybir.dt.float32
    i8 = mybir.dt.int8
    not_ = nb * Nw // 128
    ntc = not_ // NWCHUNK
    nto = nb * No // 128
    ins = {
        "xoq": nc.dram_tensor("xoq", [nb * No, 512], i8, kind="ExternalInput").ap(),
        "xosc": nc.dram_tensor("xosc", [128, nto], f32, kind="ExternalInput").ap(),
        "wh": nc.dram_tensor("wh", [512, 1024], f16, kind="ExternalInput").ap(),
        **{f"xwq{i}": nc.dram_tensor(f"xwq{i}", [ntc * 128, 512], i8,
                                     kind="ExternalInput").ap()
           for i in range(NWCHUNK)},
        **{f"xwsc{i}": nc.dram_tensor(f"xwsc{i}", [128, ntc], f32,
                                      kind="ExternalInput").ap()
           for i in range(NWCHUNK)},
        "swp": nc.dram_tensor("swp", [128, nb * 8], f16, kind="ExternalInput").ap(),
        "sA2": nc.dram_tensor("sA2", [1, nb * 148], f16, kind="ExternalInput").ap(),
    }
    if has_bias:
        ins["bias128"] = nc.dram_tensor("bias128", [128, 4], f32,
                                        kind="ExternalInput").ap()
        ins["biasrow"] = nc.dram_tensor("biasrow", [1, 512], f16,
                                        kind="ExternalInput").ap()
    out_ap = nc.dram_tensor("out", [nb, 512], f32, kind="ExternalOutput").ap()
    with tile.TileContext(nc) as tc:
        build_gat(tc, out_ap, ins, nb=nb, nblk=nblk, has_bias=has_bias)
    nc.compile()
    _built[key] = nc
    return nc


# ---- host-side packing (jax cpu jit, multithreaded) ----

_prep_jit = None


def _get_prep_jit():
    global _prep_jit
    if _prep_jit is not None:
        return _prep_jit
    import functools
    import jax
    import jax.numpy as jnp

    cpu = jax.devices("cpu")[0]

    @functools.partial(jax.jit, static_argnums=(1,))
    def _prep_wchunk(word_embs, i):
        # quantize word rows of upload chunk i: per-core rows
        # [i*rows_pc, (i+1)*rows_pc) with rows_pc = nb*Nw/NWCHUNK
        B = word_embs.shape[0]
        nb = B // NCORES
        rows_pc = nb * Nw // NWCHUNK
        ntc = rows_pc // 128
        wf = word_embs.reshape(NCORES, NWCHUNK, rows_pc, D)[:, i]
        wf = wf.reshape(NCORES * rows_pc, D)
        # per-row scale = 4.4x RMS of a 128-col sample (clipped below): the
        # sum-of-squares reduce vectorizes far better than an amax pass here
        wam = jnp.maximum(
            jnp.sqrt(jnp.mean(wf[:, :128] * wf[:, :128], axis=1)) * 4.4, 1e-20)
        ws = wam / 127.0
        q = jnp.clip(jnp.rint(wf * (1.0 / ws)[:, None]), -127, 127).astype(jnp.int8)
        sc = ws.reshape(NCORES, ntc, 128).transpose(0, 2, 1).reshape(
            NCORES * 128, ntc).astype(jnp.float32)
        return q, sc

    @jax.jit
    def _prep_rest(word_embs, object_embs, W, att_src, att_dst):
        B = word_embs.shape[0]
        nb = B // NCORES
        nbl = nb // _NBLK
        wf = word_embs.reshape(B * Nw, D)
        of = object_embs.reshape(B * No, D)
        oam = jnp.maximum(
            jnp.sqrt(jnp.mean(of[:, :128] * of[:, :128], axis=1)) * 4.4, 1e-20)
        osc = oam / 127.0
        xoq = jnp.clip(jnp.rint(of * (1.0 / osc)[:, None]), -127, 127).astype(jnp.int8)
        nto = nb * No // 128
        xosc = osc.reshape(NCORES, nto, 128).transpose(0, 2, 1).reshape(
            NCORES * 128, nto).astype(jnp.float32)
        # exact attention scores
        Wr = W.reshape(D, H, E)
        wa_src = jnp.einsum('dhe,he->dh', Wr, att_src)
        wa_dst = jnp.einsum('dhe,he->dh', Wr, att_dst)
        waf = jnp.concatenate([wa_src, wa_dst], axis=1)  # [D, 4]
        s_w = wf @ waf   # [B*Nw, 4]
        s_o = of @ wa_src  # [B*No, 2]
        # swp [core*128, nb*8]: col = blk*nwt*4 + wt*4 + f ; row ~ (core, p)
        nwt = _NBLK * 2
        swp = s_w.reshape(NCORES, nbl, nwt, 128, 4).transpose(0, 3, 1, 2, 4)
        swp = swp.reshape(NCORES * 128, nb * 8).astype(jnp.float16)
        # sA2 [core, nb*148]: col = b*148 + whi*74 + h*37 + n
        so = s_o.reshape(NCORES, nb, No, H).transpose(0, 1, 3, 2)  # [c, b, h, n]
        sA2 = jnp.zeros((NCORES, nb, 2, H, 37), jnp.float16)
        sA2 = sA2.at[:, :, :, :, 0:No].set(
            so[:, :, None, :, :].astype(jnp.float16))
        sA2 = sA2.reshape(NCORES, nb * 148)
        # wh replicated
        wh = jnp.tile(W.astype(jnp.float16), (NCORES, 1))
        return xoq, xosc, wh, swp, sA2

    _prep_jit = (_prep_wchunk, _prep_rest, cpu)
    return _prep_jit


# ---- cached PJRT dispatch (one jit closure per build, reused warm) ----

_disp = {}


def _get_disp(nb, nblk, has_bias):
    key = (nb, nblk, has_bias)
    if key in _disp:
        return _disp[key]
    import jax
    from jax.sharding import Mesh, PartitionSpec, NamedSharding
    from jax.experimental.shard_map import shard_map
    from concourse import bass2jax

    nc = _build(nb, nblk, has_bias)
    bass2jax.install_neuronx_cc_hook()
    assert nc.dbg_addr is None or not nc.dbg_callbacks
    partition_name = nc.partition_id_tensor.name if nc.partition_id_tensor else None

    in_names, out_names, out_avals, out_shapes = [], [], [], []
    for alloc in nc.m.functions[0].allocations:
        if not isinstance(alloc, mybir.MemoryLocationSet):
            continue
        name = alloc.memorylocations[0].name
        if alloc.kind == "ExternalInput":
            if name != partition_name:
                in_names.append(name)
        elif alloc.kind == "ExternalOutput":
            shape = tuple(alloc.tensor_shape)
            dtype = mybir.dt.np(alloc.dtype)
            out_names.append(name)
            out_avals.append(jax.core.ShapedArray(shape, dtype))
            out_shapes.append((shape, dtype))
    n_params = len(in_names)
    n_outs = len(out_avals)
    in_names_all = list(in_names) + list(out_names)
    if partition_name is not None:
        in_names_all.append(partition_name)
    extra = []
    if nc.dbg_addr is not None:
        in_names_all.append(nc.dbg_addr.name)
        extra.append(np.zeros((1, 2), np.uint32))

    donate = tuple(range(n_params, n_params + n_outs))

    def _body(*args):
        operands = list(args)
        if partition_name is not None:
            operands.append(bass2jax.partition_id_tensor())
        outs = bass2jax._bass_exec_p.bind(
            *operands,
            out_avals=tuple(out_avals),
            in_names=tuple(in_names_all),
            out_names=tuple(out_names),
            lowering_input_output_aliases=(),
            sim_require_finite=True,
            sim_require_nnan=True,
            nc=nc,
        )
        return tuple(outs)

    devices = jax.devices()[:NCORES]
    mesh = Mesh(np.asarray(devices), ("core",))
    nargs = n_params + n_outs + len(extra)
    in_specs = (PartitionSpec("core"),) * nargs
    out_specs = (PartitionSpec("core"),) * n_outs
    sharded = jax.jit(
        shard_map(_body, mesh=mesh, in_specs=in_specs, out_specs=out_specs,
                  check_rep=False),
        donate_argnums=donate, keep_unused=True,
    )
    sh = NamedSharding(mesh, PartitionSpec("core"))
    d = {
        "sharded": sharded, "sharding": sh, "in_names": in_names,
        "out_shapes": out_shapes, "extra": extra,
    }
    _disp[key] = d
    return d


def _run(inputs, trace=False):
    import jax

    object_embs = np.asarray(inputs["object_embs"], np.float32)
    word_embs = np.asarray(inputs["word_embs"], np.float32)
    W = np.asarray(inputs["W"], np.float32)
    att_src = np.asarray(inputs["att_src"], np.float32)
    att_dst = np.asarray(inputs["att_dst"], np.float32)
    bias = np.asarray(inputs["bias"], np.float32)
    has_bias = bool(np.any(bias))
    B = object_embs.shape[0]
    nb = B // NCORES

    (prep_wchunk, prep_rest, cpu) = _get_prep_jit()
    if trace:
        # profiling path: per-core in_maps through run_bass_kernel_spmd
        with jax.default_device(cpu):
            chunks = [prep_wchunk(word_embs, i) for i in range(NWCHUNK)]
            rest = prep_rest(word_embs, object_embs, W, att_src, att_dst)
        host = {}
        for i, (q, sc) in enumerate(chunks):
            host[f"xwq{i}"] = np.asarray(q)
            host[f"xwsc{i}"] = np.asarray(sc)
        for k, v in zip(["xoq", "xosc", "wh", "swp", "sA2"], rest):
            host[k] = np.asarray(v)
        if has_bias:
            host["bias128"] = np.tile(
                np.ascontiguousarray(bias.reshape(4, 128).T.astype(np.float32)),
                (NCORES, 1))
            host["biasrow"] = np.tile(
                bias.reshape(1, 512).astype(np.float16), (NCORES, 1))
        from concourse import bass_utils
        nc = _build(nb, _NBLK, has_bias)
        in_maps = []
        for core in range(NCORES):
            m = {}
            for k, v in host.items():
                rows = v.shape[0] // NCORES
                m[k] = np.ascontiguousarray(v[core * rows:(core + 1) * rows])
            in_maps.append(m)
        res = bass_utils.run_bass_kernel_spmd(
            nc, in_maps, core_ids=list(range(NCORES)), trace=True)
        out = np.concatenate([r["out"] for r in res.results], axis=0)
        return out, res

    d = _get_disp(nb, _NBLK, has_bias)
    sh = d["sharding"]
    puts = {}
    with jax.default_device(cpu):
        # dispatch all host prep asynchronously (XLA-CPU queues them in order)
        chunks = [prep_wchunk(word_embs, i) for i in range(NWCHUNK)]
        rest = prep_rest(word_embs, object_embs, W, att_src, att_dst)
    # as each chunk's quantize completes, start its upload; the axon link
    # streams in the background while later chunks still compute
    for i, (q, sc) in enumerate(chunks):
        puts[f"xwq{i}"] = jax.device_put(np.asarray(q), sh)
        puts[f"xwsc{i}"] = jax.device_put(np.asarray(sc), sh)
    for k, v in zip(["xoq", "xosc", "wh", "swp", "sA2"], rest):
        puts[k] = jax.device_put(np.asarray(v), sh)
    if has_bias:
        puts["bias128"] = jax.device_put(np.tile(
            np.ascontiguousarray(bias.reshape(4, 128).T.astype(np.float32)),
            (NCORES, 1)), sh)
        puts["biasrow"] = jax.device_put(np.tile(
            bias.reshape(1, 512).astype(np.float16), (NCORES, 1)), sh)
    args = [puts[k] for k in d["in_names"]]
    zeros = [np.zeros((NCORES * s[0], *s[1:]), dt) for (s, dt) in d["out_shapes"]]
    out_arrs = d["sharded"](*args, *zeros, *d["extra"])
    out = np.asarray(out_arrs[0])
    return out, None


def kernel(**inputs) -> np.ndarray:
    return _run(inputs, trace=False)[0]


# revision 16
# speedup vs baseline: 1.0272x; 1.0272x over previous
"""Trainium2 Bass kernel for nn_ObjectWordGAT (8-core data parallel).

Self-contained: accepts FULL inputs, shards batch across 8 NeuronCores,
returns the FULL [256, 512] fp32 output.

Warm-path design (the wall clock is dominated by the ~73MB/s axon link and
~0.1s RPC latencies, not HW exec):
  - embeddings are uploaded as per-row int8 (natural row-major layout), and
    dequantized + transposed on-chip (scalar engine affine + PE transposes),
    eliminating both the host-side transpose and half the upload bytes;
  - attention scores s = x @ (W @ att) are computed exactly on host (tiny
    sgemm) and uploaded pre-packed (~0.6MB), removing the quantization error
    from the logit path;
  - wm (head-mean projection) is derived on-chip from wh;
  - the jitted shard_map dispatch closure is built once and cached, so warm
    calls skip retracing/recompiling;
  - all device_puts are issued asynchronously and overlap each other.
"""
import numpy as np
import concourse.mybir as mybir


# ======== gat_core.py ========

from contextlib import ExitStack

from concourse.masks import make_identity

F16 = mybir.dt.float16
F32 = mybir.dt.float32
I8 = mybir.dt.int8
AF = mybir.ActivationFunctionType
ALU = mybir.AluOpType
AX = mybir.AxisListType

D = 512
H = 2
E = 512
No = 36
Nw = 256
NEG = 0.2
NWCHUNK = 4  # word rows are uploaded in this many pipelined chunks


def build_gat(tc, out_ap, ins, nb=32, nblk=4, has_bias=False, dbg=None):
    def tap(name, ap):
        if dbg is not None and name in dbg:
            tc.nc.sync.dma_start(dbg[name][:], ap)

    nc = tc.nc
    xoq, xosc = ins["xoq"], ins["xosc"]
    wh, swp, sA2 = ins["wh"], ins["swp"], ins["sA2"]
    RW, RO = nb * Nw, nb * 64  # obj rows padded to 64 per b
    ROP = nb * No              # packed obj rows (36 per b)
    nbl = nb // nblk
    not_ = nb * Nw // 128      # word row tiles per core
    ntc = not_ // NWCHUNK      # word row tiles per upload chunk
    nto = ROP // 128           # obj row tiles per core (1152/128 = 9)
    assert ROP % 128 == 0
    assert nb % nblk == 0 and nblk % 2 == 0

    ctx = ExitStack()
    with ctx:
        const = ctx.enter_context(tc.tile_pool(name="const", bufs=1))
        # ---- constants ----
        wh_sb = [const.tile([128, 1024], F16, name=f"wh{c}", tag=f"wh{c}") for c in range(4)]
        wm_sb = [const.tile([128, 512], F16, name=f"wm{c}", tag=f"wm{c}") for c in range(4)]
        for c in range(4):
            sl = slice(c * 128, (c + 1) * 128)
            nc.sync.dma_start(wh_sb[c][:], wh[sl, :])
        ident16 = const.tile([128, 128], F16, name="id16", tag="id16")
        ident32 = const.tile([128, 128], F32, name="id32", tag="id32")
        make_identity(nc, ident16[:])
        make_identity(nc, ident32[:])
        ones16 = const.tile([1, 128], F16, name="ones16", tag="ones16")
        nc.vector.memset(ones16[:], 1.0)
        # wm = 0.5 * (wh_head0 + wh_head1), on-chip
        for c in range(4):
            nc.vector.tensor_add(wm_sb[c][:], wh_sb[c][:, 0:512], wh_sb[c][:, 512:1024])
            nc.scalar.mul(wm_sb[c][:], wm_sb[c][:], 0.5)
        # packed attention-score constants (computed on host, exact)
        swp_sb = const.tile([128, nbl * nblk * 8], F16, name="swp", tag="swp")
        nc.sync.dma_start(swp_sb[:], swp[:, :])
        sA2_sb = const.tile([1, nb * 148], F16, name="sA2", tag="sA2")
        nc.sync.dma_start(sA2_sb[:], sA2[:, :])
        # quant scales
        xwsc_sb = [const.tile([128, ntc], F32, name=f"xwsc{i}", tag=f"xwsc{i}")
                   for i in range(NWCHUNK)]
        for i in range(NWCHUNK):
            nc.sync.dma_start(xwsc_sb[i][:], ins[f"xwsc{i}"][:, :])
        xosc_sb = const.tile([128, nto], F32, name="xosc", tag="xosc")
        nc.sync.dma_start(xosc_sb[:], xosc[:, :])
        if has_bias:
            bias_sb = const.tile([128, 4], F32, name="bias128", tag="bias128")
            nc.sync.dma_start(bias_sb[:], ins["bias128"][:, :])
            biasrow_sb = const.tile([1, 512], F16, name="biasrow", tag="biasrow")
            nc.sync.dma_start(biasrow_sb[:], ins["biasrow"][:, :])

        # resident results
        ngrp2 = nb // 2  # obj rows padded: 2 b per 128-row tile
        xto_sb = [const.tile([128, RO], F16, name=f"xto{c}", tag=f"xto{c}") for c in range(4)]
        hobj_sb = const.tile([128, ngrp2 * 1024], F16, name="hobj", tag="hobj")
        uoT_sb = const.tile([128, 4 * RO], F16, name="uoT", tag="uoT")
        outT_sb = const.tile([128, nb * 4], F32, name="outT", tag="outT")

        # ================= PHASE 0: objects dequant + transpose =================
        # xoq [ROP, 512] int8 natural -> xto_sb[c] [128, RO] f16 (64-padded per b)
        with (
            tc.tile_pool(name="sb0", bufs=2) as sb0,
            tc.tile_pool(name="ps0", bufs=2, space="PSUM") as ps0,
        ):
            xtoP_sb = [const.tile([128, ROP], F16, name=f"xtoP{c}", tag=f"xtoP{c}")
                       for c in range(4)]
            for t in range(nto):
                xq_t = sb0.tile([128, 512], I8, name="xq", tag="xq")
                nc.sync.dma_start(xq_t[:], xoq[t * 128:(t + 1) * 128, :])
                xf_t = sb0.tile([128, 512], F16, name="xf", tag="xf")
                nc.scalar.mul(xf_t[:], xq_t[:], xosc_sb[:, t:t + 1])
                pt = ps0.tile([128, 512], F16, name="pt", tag="pt")
                for c in range(4):
                    nc.tensor.transpose(pt[:, c * 128:(c + 1) * 128],
                                        xf_t[:, c * 128:(c + 1) * 128], ident16[:])
                for c in range(4):
                    eng = nc.scalar.copy if (t + c) % 2 == 0 else nc.vector.tensor_copy
                    eng(xtoP_sb[c][:, t * 128:(t + 1) * 128],
                        pt[:, c * 128:(c + 1) * 128])
            # pad 36 -> 64 per b
            for c in range(4):
                nc.gpsimd.memset(
                    xto_sb[c][:].rearrange("p (b n) -> p b n", n=64)[:, :, No:64], 0.0)
                nc.vector.tensor_copy(
                    xto_sb[c][:].rearrange("p (b n) -> p b n", n=64)[:, :, 0:No],
                    xtoP_sb[c][:].rearrange("p (b n) -> p b n", n=No))
        tap("xto0", xto_sb[0][:])

        # ================= PHASE A: objects =================
        with tc.tile_pool(name="psA", bufs=2, space="PSUM") as psA:
            for g in range(ngrp2):
                pt = psA.tile([128, 1024], F32, name="phobj", tag="phobj")
                for he in range(2):
                    for c in range(4):
                        nc.tensor.matmul(
                            pt[:, he * 512:(he + 1) * 512],
                            lhsT=xto_sb[c][:, 128 * g:128 * (g + 1)],
                            rhs=wh_sb[c][:, he * 512:(he + 1) * 512],
                            start=(c == 0), stop=(c == 3),
                        )
                eng = nc.scalar.copy if g % 2 == 0 else nc.vector.tensor_copy
                eng(hobj_sb[:, g * 1024:(g + 1) * 1024], pt[:, :])

        with tc.tile_pool(name="psB", bufs=2, space="PSUM") as psB:
            # upd_obj^T = Wm.T @ Xo^T (+bias on evac)
            nchunks = [(i, min(512, RO - i)) for i in range(0, RO, 512)]
            for ec in range(4):
                for n0, nn in nchunks:
                    pt = psB.tile([128, 512], F32, name="puoT", tag="puoT")
                    for c in range(4):
                        nc.tensor.matmul(
                            pt[:, 0:nn],
                            lhsT=wm_sb[c][:, ec * 128:(ec + 1) * 128],
                            rhs=xto_sb[c][:, n0:n0 + nn],
                            start=(c == 0), stop=(c == 3),
                        )
                    dst = uoT_sb[:, ec * RO + n0: ec * RO + n0 + nn]
                    if has_bias:
                        nc.scalar.activation(dst, pt[:, 0:nn], AF.Identity,
                                             bias=bias_sb[:, ec:ec + 1])
                    elif (ec * len(nchunks) + n0 // 512) % 2 == 0:
                        nc.scalar.copy(dst, pt[:, 0:nn])
                    else:
                        nc.vector.tensor_copy(dst, pt[:, 0:nn])

        tap("hobj", hobj_sb[:])
        tap("uoT", uoT_sb[:])
        tap("sA2", sA2_sb[:])

        # ================= PHASE B: word blocks =================
        with (
            tc.tile_pool(name="sbB", bufs=2) as sbB,
            tc.tile_pool(name="ps_hw", bufs=2, space="PSUM") as ps_hw,
            tc.tile_pool(name="ps_mid", bufs=2, space="PSUM") as ps_mid,
            tc.tile_pool(name="ps_sm", bufs=2, space="PSUM") as ps_sm,
            tc.tile_pool(name="ps_aT", bufs=1, space="PSUM") as ps_aT,
        ):
            for blk in range(nbl):
                gw0 = blk * nblk * Nw  # first word row of block
                nwt = nblk * 2  # 128-row word tiles in block
                ng = nblk * 4   # (bi, whi, h) groups in block
                # ---- dequant + transpose words of this block ----
                xtw_sb = [sbB.tile([128, nblk * 256], F16, name=f"xtw{c}", tag=f"xtw{c}")
                          for c in range(4)]
                for wt8 in range(nwt):
                    t = blk * nwt + wt8
                    ch, tci = t // ntc, t % ntc
                    xq_t = sbB.tile([128, 512], I8, name="xqw", tag="xqw")
                    nc.sync.dma_start(
                        xq_t[:], ins[f"xwq{ch}"][tci * 128:(tci + 1) * 128, :])
                    xf_t = sbB.tile([128, 512], F16, name="xfw", tag="xfw")
                    nc.scalar.mul(xf_t[:], xq_t[:], xwsc_sb[ch][:, tci:tci + 1])
                    pt = ps_aT.tile([128, 512], F16, name="paT", tag="paT")
                    for c in range(4):
                        nc.tensor.transpose(pt[:, c * 128:(c + 1) * 128],
                                            xf_t[:, c * 128:(c + 1) * 128], ident16[:])
                    for c in range(4):
                        eng = nc.scalar.copy if (wt8 + c) % 2 == 0 else nc.vector.tensor_copy
                        eng(xtw_sb[c][:, wt8 * 128:(wt8 + 1) * 128],
                            pt[:, c * 128:(c + 1) * 128])

                # ---- s_word: packed slice of the host-computed scores ----
                sw_sb = swp_sb[:, blk * nwt * 4:(blk + 1) * nwt * 4]
                if blk == 0:
                    tap("sw", sw_sb)

                # ---- spread [128, nblk*148] = s_dst col per (bi,whi,h) ----
                spread_sb = sbB.tile([128, nblk * 148], F16, name="spread", tag="spread")
                src = sw_sb.rearrange("p (b whi f) -> p b whi f",
                                      b=nblk, whi=2)[:, :, :, 2:4]
                src = src.broadcast_to([128, nblk, 2, 2, 37])
                dst = spread_sb[:].rearrange("p (b whi h n) -> p b whi h n",
                                             b=nblk, whi=2, h=2)
                nc.vector.tensor_copy(dst, src)
                # self column (n=36): s_src + s_dst
                swg = sw_sb.rearrange("p (b whi f) -> p b whi f",
                                      b=nblk, whi=2)
                nc.vector.tensor_add(
                    dst[:, :, :, :, 36:37].rearrange("p b whi h n -> p b whi (h n)"),
                    dst[:, :, :, :, 36:37].rearrange("p b whi h n -> p b whi (h n)"),
                    swg[:, :, :, 0:2])

                # ---- L psums + lrelu + exp ----
                L2_sb = sbB.tile([128, nblk * 148], F32, name="L2", tag="L2")
                half = nblk * 148 // 2
                for hf in range(2):
                    p_L = ps_sm.tile([128, half], F32, name="sm", tag="sm")
                    nc.tensor.matmul(
                        p_L[:], lhsT=ones16[:],
                        rhs=sA2_sb[0:1, blk * nblk * 148 + hf * half:][:, 0:half],
                        start=True, stop=False)
                    nc.tensor.matmul(
                        p_L[:], lhsT=ident16[:],
                        rhs=spread_sb[:, hf * half:(hf + 1) * half],
                        start=False, stop=True)
                    ltmp = sbB.tile([128, half], F16, name="ltmp", tag="ltmp")
                    nc.scalar.mul(ltmp[:], p_L[:], NEG)
                    nc.vector.tensor_max(
                        L2_sb[:, hf * half:(hf + 1) * half], p_L[:], ltmp[:])
                expL_sb = sbB.tile([128, nblk * 148], F32, name="expL", tag="expL")
                nc.scalar.activation(expL_sb[:], L2_sb[:], AF.Exp)
                if blk == 0:
                    tap("L2", L2_sb[:])
                    tap("expL", expL_sb[:])

                # ---- den, r, alpha, c ----
                expg = expL_sb[:].rearrange("p (g n) -> p g n", n=37)
                den_sb = sbB.tile([128, ng], F32, name="den", tag="den")
                nc.vector.tensor_reduce(den_sb[:], expg, axis=AX.X, op=ALU.add)
                r_sb = sbB.tile([128, ng], F32, name="r", tag="r")
                nc.vector.reciprocal(r_sb[:], den_sb[:])
                nc.vector.tensor_scalar_mul(r_sb[:], r_sb[:], 0.5)
                alpha_sb = sbB.tile([128, ng * 64], F16, name="alpha", tag="alpha")
                nc.gpsimd.memset(
                    alpha_sb[:].rearrange("p (g n) -> p g n", n=64)[:, :, 36:64],
                    0.0)
                rbc = r_sb[:].broadcast_to([128, ng, 36])
                nc.vector.tensor_mul(
                    alpha_sb[:].rearrange("p (g n) -> p g n", n=64)[:, :, 0:36],
                    expg[:, :, 0:36], rbc)
                c_sb = sbB.tile([128, ng], F32, name="c", tag="c")
                nc.vector.tensor_mul(
                    c_sb[:],
                    expg[:, :, 36:37].rearrange("p g n -> p (g n)"), r_sb[:])
                if blk == 0:
                    tap("den", den_sb[:])
                    tap("alpha", alpha_sb[:])
                    tap("c", c_sb[:])

                # ---- alpha transposes -> aT [128, (nblk/2)*512] ----
                # partition half = b parity; col = pair*512 + h*256 + whi*128
                aT_sb = sbB.tile([128, (nblk // 2) * 512], F16, name="aT",
                                 tag="aT")
                for pr in range(nblk // 2):
                    p_aTt = ps_aT.tile([128, 512], F16, name="paT", tag="paT")
                    for pb in range(2):
                        bi = pr * 2 + pb
                        for whi in range(2):
                            for h in range(2):
                                g = (bi * 2 + whi) * 2 + h
                                nc.tensor.transpose(
                                    p_aTt[64 * pb:64 * pb + 64,
                                          h * 256 + whi * 128:][:, 0:128],
                                    alpha_sb[:, g * 64:(g + 1) * 64],
                                    ident16[:],
                                    tile_position=(0, 64 * pb),
                                )
                    nc.vector.tensor_copy(aT_sb[:, pr * 512:(pr + 1) * 512],
                                          p_aTt[:])

                if blk == 0:
                    tap("aT", aT_sb[:])
                # ---- h_word proj + t + msg + uw per (bi, whi) ----
                t_sb = sbB.tile([128, nwt * 512], F16, name="t", tag="t")
                uw_sb = sbB.tile([128, nwt * 512], F16, name="uw", tag="uw")
                for bi in range(nblk):
                    b = blk * nblk + bi
                    for whi in range(2):
                        wt = bi * 2 + whi
                        g = wt * 2  # (bi, whi, h=0)
                        p_he0 = ps_hw.tile([128, 512], F32, name="hw", tag="hw")
                        for c in range(4):
                            nc.tensor.matmul(
                                p_he0[:],
                                lhsT=xtw_sb[c][:, wt * 128:(wt + 1) * 128],
                                rhs=wh_sb[c][:, 0:512],
                                start=(c == 0), stop=(c == 3))
                        t0_sb = sbB.tile([128, 512], F16, name="t0", tag="t0")
                        nc.scalar.mul(t0_sb[:], p_he0[:], c_sb[:, g:g + 1])
                        p_he1 = ps_hw.tile([128, 512], F32, name="hw", tag="hw")
                        for c in range(4):
                            nc.tensor.matmul(
                                p_he1[:],
                                lhsT=xtw_sb[c][:, wt * 128:(wt + 1) * 128],
                                rhs=wh_sb[c][:, 512:1024],
                                start=(c == 0), stop=(c == 3))
                        t1_sb = sbB.tile([128, 512], F16, name="t1", tag="t1")
                        nc.vector.tensor_scalar_mul(t1_sb[:], p_he1[:],
                                                    c_sb[:, g + 1:g + 2])
                        nc.gpsimd.tensor_add(t_sb[:, wt * 512:(wt + 1) * 512],
                                             t0_sb[:], t1_sb[:])

                        # msg: two K=36 matmuls at row base 64*(b%2)
                        p_msg = ps_mid.tile([128, 512], F32, name="mid", tag="mid")
                        gq, go = b // 2, 64 * (b % 2)
                        acol = (bi // 2) * 512 + whi * 128
                        nc.tensor.matmul(
                            p_msg[:],
                            lhsT=aT_sb[go:go + 36, acol:acol + 128],
                            rhs=hobj_sb[go:go + 36, gq * 1024:gq * 1024 + 512],
                            start=True, stop=False,
                            tile_position=(go, 0))
                        nc.tensor.matmul(
                            p_msg[:],
                            lhsT=aT_sb[go:go + 36, acol + 256:acol + 256 + 128],
                            rhs=hobj_sb[go:go + 36,
                                        gq * 1024 + 512:gq * 1024 + 1024],
                            start=False, stop=not has_bias,
                            tile_position=(go, 0))
                        if has_bias:
                            nc.tensor.matmul(p_msg[:], lhsT=ones16[:],
                                             rhs=biasrow_sb[:],
                                             start=False, stop=True)
                        nc.vector.tensor_add(
                            uw_sb[:, wt * 512:(wt + 1) * 512], p_msg[:],
                            t_sb[:, wt * 512:(wt + 1) * 512])

                if blk == 0:
                    tap("t", t_sb[:])
                    tap("uw", uw_sb[:])
                # ---- uw transposes -> uwT [128, nblk*4*256] ----
                uwT_sb = sbB.tile([128, nblk * 4 * 256], F16, name="uwT", tag="uwT")
                for bi in range(nblk):
                    for ec in range(4):
                        p_uwT = ps_mid.tile([128, 256], F16, name="mid", tag="mid")
                        for whi in range(2):
                            nc.tensor.transpose(
                                p_uwT[:, whi * 128:(whi + 1) * 128],
                                uw_sb[:, (bi * 2 + whi) * 512 + ec * 128:][:, 0:128],
                                ident16[:])
                        dst = uwT_sb[:, (bi * 4 + ec) * 256:
                                     (bi * 4 + ec + 1) * 256]
                        if ec % 2 == 0:
                            nc.scalar.copy(dst, p_uwT[:])
                        else:
                            nc.vector.tensor_copy(dst, p_uwT[:])

                if blk == 0:
                    tap("uwT", uwT_sb[:])
                # ---- C + softmax + attnT ----
                p_attnT = ps_aT.tile([128, nblk * 2 * 36], F16, name="pattnT", tag="pattnT")
                for pair in range(nblk // 2):
                    p_C = ps_sm.tile([128, 256], F32, name="sm", tag="sm")
                    for pb in range(2):
                        bi = pair * 2 + pb
                        b = blk * nblk + bi
                        for ec in range(4):
                            nc.tensor.matmul(
                                p_C[64 * pb:64 * pb + 36, :],
                                lhsT=uoT_sb[:, ec * RO + b * 64:
                                            ec * RO + b * 64 + 36],
                                rhs=uwT_sb[:, (bi * 4 + ec) * 256:
                                           (bi * 4 + ec + 1) * 256],
                                start=(ec == 0), stop=(ec == 3),
                                tile_position=(0, 64 * pb))
                    negmax = sbB.tile([128, 1], F32, name="negmax", tag="negmax")
                    expC = sbB.tile([128, 256], F16, name="expC", tag="expC")
                    den2 = sbB.tile([128, 1], F32, name="den2", tag="den2")
                    rden = sbB.tile([128, 1], F32, name="rden", tag="rden")
                    attn = sbB.tile([128, 256], F16, name="attn", tag="attn")
                    for pb in range(2):
                        rs = slice(64 * pb, 64 * pb + 36)
                        nc.vector.tensor_reduce(negmax[rs], p_C[rs, :], axis=AX.X,
                                                op=ALU.max, negate=True)
                        nc.scalar.activation(expC[rs, :], p_C[rs, :], AF.Exp,
                                             bias=negmax[rs], accum_out=den2[rs])
                        nc.vector.reciprocal(rden[rs], den2[rs])
                        nc.vector.tensor_scalar_mul(rden[rs], rden[rs],
                                                    1.0 / 36.0)
                        nc.vector.tensor_scalar_mul(attn[rs, :], expC[rs, :],
                                                    rden[rs])
                    if blk == 0 and pair == 0:
                        tap("attn", attn[:])
                        tap("expC", expC[:])
                    for pb in range(2):
                        bi = pair * 2 + pb
                        for whi in range(2):
                            nc.tensor.transpose(
                                p_attnT[:, (bi * 2 + whi) * 36:
                                        (bi * 2 + whi + 1) * 36],
                                attn[64 * pb:64 * pb + 36,
                                     whi * 128:(whi + 1) * 128],
                                ident16[64 * pb:64 * pb + 36,
                                        64 * pb:64 * pb + 36],
                                tile_position=(64 * pb, 0))
                attnT_sb = sbB.tile([128, nblk * 2 * 36], F16, name="attnT", tag="attnT")
                nc.vector.tensor_copy(attnT_sb[:], p_attnT[:])
                if blk == 0:
                    tap("attnT", attnT_sb[:])

                # ---- weighted^T + final reduce ----
                for bi in range(nblk):
                    b = blk * nblk + bi
                    p_w = ps_sm.tile([128, 144], F32, name="sm", tag="sm")
                    for ec in range(4):
                        for whi in range(2):
                            nc.tensor.matmul(
                                p_w[:, ec * 36:(ec + 1) * 36],
                                lhsT=uw_sb[:, (bi * 2 + whi) * 512 +
                                           ec * 128:][:, 0:128],
                                rhs=attnT_sb[:, (bi * 2 + whi) * 36:
                                             (bi * 2 + whi + 1) * 36],
                                start=(whi == 0), stop=(whi == 1))
                    nc.vector.tensor_reduce(
                        outT_sb[:, b * 4:(b + 1) * 4],
                        p_w[:].rearrange("p (ec n) -> p ec n", n=36),
                        axis=AX.X, op=ALU.add)

        tap("outT", outT_sb[:])
        # ================= PHASE C: final transpose + store =================
        with tc.tile_pool(name="psC", bufs=1, space="PSUM") as psC:
            assert nb <= 128
            p_out = psC.tile([nb, 512], F32, name="p_out", tag="p_out")
            for ec in range(4):
                src = outT_sb[:].rearrange("p (b ec) -> p ec b", ec=4)[:, ec, :]
                nc.tensor.transpose(p_out[0:nb, ec * 128:(ec + 1) * 128],
                                    src, ident32[:])
            out_sb = const.tile([nb, 512], F32, name="out_sb", tag="out_sb")
            nc.vector.tensor_copy(out_sb[0:nb, :], p_out[0:nb, :])
            nc.sync.dma_start(out_ap[:, :], out_sb[0:nb, :])


# ======== runner.py ========

NCORES = 8
_B_TOTAL = 256
_NB = _B_TOTAL // NCORES  # 32
_NBLK = 4

_built = {}


def _build(nb, nblk, has_bias):
    key = (nb, nblk, has_bias)
    if key in _built:
        return _built[key]
    import concourse.bacc as bacc
    import concourse.tile as tile

    nc = bacc.Bacc(trn_type="TRN2", target_bir_lowering=False, debug=False,
                   num_devices=NCORES)
    f16 = mybir.dt.float16
    f32 = mybir.dt.float32
    i8 = mybir.dt.int8
    not_ = nb * Nw // 128
    ntc = not_ // NWCHUNK
    nto = nb * No // 128
    ins = {
        "xoq": nc.dram_tensor("xoq", [nb * No, 512], i8, kind="ExternalInput").ap(),
        "xosc": nc.dram_tensor("xosc", [128, nto], f32, kind="ExternalInput").ap(),
        "wh": nc.dram_tensor("wh", [512, 1024], f16, kind="ExternalInput").ap(),
        **{f"xwq{i}": nc.dram_tensor(f"xwq{i}", [ntc * 128, 512], i8,
                                     kind="ExternalInput").ap()
           for i in range(NWCHUNK)},
        **{f"xwsc{i}": nc.dram_tensor(f"xwsc{i}", [128, ntc], f32,
                                      kind="ExternalInput").ap()
           for i in range(NWCHUNK)},
        "swp": nc.dram_tensor("swp", [128, nb * 8], f16, kind="ExternalInput").ap(),
        "sA2": nc.dram_tensor("sA2", [1, nb * 148], f16, kind="ExternalInput").ap(),
    }
    if has_bias:
        ins["bias128"] = nc.dram_tensor("bias128", [128, 4], f32,
                                        kind="ExternalInput").ap()
        ins["biasrow"] = nc.dram_tensor("biasrow", [1, 512], f16,
                                        kind="ExternalInput").ap()
    out_ap = nc.dram_tensor("out", [nb, 512], f32, kind="ExternalOutput").ap()
    with tile.TileContext(nc) as tc:
        build_gat(tc, out_ap, ins, nb=nb, nblk=nblk, has_bias=has_bias)
    nc.compile()
    _built[key] = nc
    return nc


# ---- host-side packing (jax cpu jit, multithreaded) ----

_prep_jit = None


def _get_prep_jit():
    global _prep_jit
    if _prep_jit is not None:
        return _prep_jit
    import functools
    import jax
    import jax.numpy as jnp

    cpu = jax.devices("cpu")[0]

    @functools.partial(jax.jit, static_argnums=(1,))
    def _prep_wchunk(word_embs, i):
        # quantize word rows of upload chunk i: per-core rows
        # [i*rows_pc, (i+1)*rows_pc) with rows_pc = nb*Nw/NWCHUNK
        B = word_embs.shape[0]
        nb = B // NCORES
        rows_pc = nb * Nw // NWCHUNK
        ntc = rows_pc // 128
        wf = word_embs.reshape(NCORES, NWCHUNK, rows_pc, D)[:, i]
        wf = wf.reshape(NCORES * rows_pc, D)
        # per-row scale = 4.4x RMS of a 128-col sample (clipped below): the
        # sum-of-squares reduce vectorizes far better than an amax pass here
        wam = jnp.maximum(
            jnp.sqrt(jnp.mean(wf[:, :128] * wf[:, :128], axis=1)) * 4.4, 1e-20)
        ws = wam / 127.0
        q = jnp.clip(jnp.rint(wf * (1.0 / ws)[:, None]), -127, 127).astype(jnp.int8)
        sc = ws.reshape(NCORES, ntc, 128).transpose(0, 2, 1).reshape(
            NCORES * 128, ntc).astype(jnp.float32)
        return q, sc

    @jax.jit
    def _prep_rest(word_embs, object_embs, W, att_src, att_dst):
        B = word_embs.shape[0]
        nb = B // NCORES
        nbl = nb // _NBLK
        wf = word_embs.reshape(B * Nw, D)
        of = object_embs.reshape(B * No, D)
        oam = jnp.maximum(
            jnp.sqrt(jnp.mean(of[:, :128] * of[:, :128], axis=1)) * 4.4, 1e-20)
        osc = oam / 127.0
        xoq = jnp.clip(jnp.rint(of * (1.0 / osc)[:, None]), -127, 127).astype(jnp.int8)
        nto = nb * No // 128
        xosc = osc.reshape(NCORES, nto, 128).transpose(0, 2, 1).reshape(
            NCORES * 128, nto).astype(jnp.float32)
        # exact attention scores
        Wr = W.reshape(D, H, E)
        wa_src = jnp.einsum('dhe,he->dh', Wr, att_src)
        wa_dst = jnp.einsum('dhe,he->dh', Wr, att_dst)
        waf = jnp.concatenate([wa_src, wa_dst], axis=1)  # [D, 4]
        s_w = wf @ waf   # [B*Nw, 4]
        s_o = of @ wa_src  # [B*No, 2]
        # swp [core*128, nb*8]: col = blk*nwt*4 + wt*4 + f ; row ~ (core, p)
        nwt = _NBLK * 2
        swp = s_w.reshape(NCORES, nbl, nwt, 128, 4).transpose(0, 3, 1, 2, 4)
        swp = swp.reshape(NCORES * 128, nb * 8).astype(jnp.float16)
        # sA2 [core, nb*148]: col = b*148 + whi*74 + h*37 + n
        so = s_o.reshape(NCORES, nb, No, H).transpose(0, 1, 3, 2)  # [c, b, h, n]
        sA2 = jnp.zeros((NCORES, nb, 2, H, 37), jnp.float16)
        sA2 = sA2.at[:, :, :, :, 0:No].set(
            so[:, :, None, :, :].astype(jnp.float16))
        sA2 = sA2.reshape(NCORES, nb * 148)
        # wh replicated
        wh = jnp.tile(W.astype(jnp.float16), (NCORES, 1))
        return xoq, xosc, wh, swp, sA2

    _prep_jit = (_prep_wchunk, _prep_rest, cpu)
    return _prep_jit


# ---- cached PJRT dispatch (one jit closure per build, reused warm) ----

_disp = {}


def _get_disp(nb, nblk, has_bias):
    key = (nb, nblk, has_bias)
    if key in _disp:
        return _disp[key]
    import jax
    from jax.sharding import Mesh, PartitionSpec, NamedSharding
    from jax.experimental.shard_map import shard_map
    from concourse import bass2jax

    nc = _build(nb, nblk, has_bias)
    bass2jax.install_neuronx_cc_hook()
    assert nc.dbg_addr is None or not nc.dbg_callbacks
    partition_name = nc.partition_id_tensor.name if nc.partition_id_tensor else None

    in_names, out_names, out_avals, out_shapes = [], [], [], []
    for alloc in nc.m.functions[0].allocations:
        if not isinstance(alloc, mybir.MemoryLocationSet):
            continue
        name = alloc.memorylocations[0].name
        if alloc.kind == "ExternalInput":
            if name != partition_name:
                in_names.append(name)
        elif alloc.kind == "ExternalOutput":
            shape = tuple(alloc.tensor_shape)
            dtype = mybir.dt.np(alloc.dtype)
            out_names.append(name)
            out_avals.append(jax.core.ShapedArray(shape, dtype))
            out_shapes.append((shape, dtype))
    n_params = len(in_names)
    n_outs = len(out_avals)
    in_names_all = list(in_names) + list(out_names)
    if partition_name is not None:
        in_names_all.append(partition_name)
    extra = []
    if nc.dbg_addr is not None:
        in_names_all.append(nc.dbg_addr.name)
        extra.append(np.zeros((1, 2), np.uint32))

    donate = tuple(range(n_params, n_params + n_outs))

    def _body(*args):
        operands = list(args)
        if partition_name is not None:
            operands.append(bass2jax.partition_id_tensor())
        outs = bass2jax._bass_exec_p.bind(
            *operands,
            out_avals=tuple(out_avals),
            in_names=tuple(in_names_all),
            out_names=tuple(out_names),
            lowering_input_output_aliases=(),
            sim_require_finite=True,
            sim_require_nnan=True,
            nc=nc,
        )
        return tuple(outs)

    devices = jax.devices()[:NCORES]
    mesh = Mesh(np.asarray(devices), ("core",))
    nargs = n_params + n_outs + len(extra)
    in_specs = (PartitionSpec("core"),) * nargs
    out_specs = (PartitionSpec("core"),) * n_outs
    sharded = jax.jit(
        shard_map(_body, mesh=mesh, in_specs=in_specs, out_specs=out_specs,
                  check_rep=False),
        donate_argnums=donate, keep_unused=True,
    )
    sh = NamedSharding(mesh, PartitionSpec("core"))
    d = {
        "sharded": sharded, "sharding": sh, "in_names": in_names,
        "out_shapes": out_shapes, "extra": extra,
    }
    _disp[key] = d
    return d


def _run(inputs, trace=False):
    import jax

    object_embs = np.asarray(inputs["object_embs"], np.float32)
    word_embs = np.asarray(inputs["word_embs"], np.float32)
    W = np.asarray(inputs["W"], np.float32)
    att_src = np.asarray(inputs["att_src"], np.float32)
    att_dst = np.asarray(inputs["att_dst"], np.float32)
    bias = np.asarray(inputs["bias"], np.float32)
    has_bias = bool(np.any(bias))
    B = object_embs.shape[0]
    nb = B // NCORES

    (prep_wchunk, prep_rest, cpu) = _get_prep_jit()
    if trace:
        # profiling path: per-core in_maps through run_bass_kernel_spmd
        with jax.default_device(cpu):
            chunks = [prep_wchunk(word_embs, i) for i in range(NWCHUNK)]
            rest = prep_rest(word_embs, object_embs, W, att_src, att_dst)
        host = {}
        for i, (q, sc) in enumerate(chunks):
            host[f"xwq{i}"] = np.asarray(q)
            host[f"xwsc{i}"] = np.asarray(sc)
        for k, v in zip(["xoq", "xosc", "wh", "swp", "sA2"], rest):
            host[k] = np.asarray(v)
        if has_bias:
            host["bias128"] = np.tile(
                np.ascontiguousarray(bias.reshape(4, 128).T.astype(np.float32)),
                (NCORES, 1))
            host["biasrow"] = np.tile(
                bias.reshape(1, 512).astype(np.float16), (NCORES, 1))
        from concourse import bass_utils
        nc = _build(nb, _NBLK, has_bias)
        in_maps = []
        for core in range(NCORES):
            m = {}
            for k, v in host.items():
                rows = v.shape[0] // NCORES
                m[k] = np.ascontiguousarray(v[core * rows:(core + 1) * rows])
            in_maps.append(m)
        res = bass_utils.run_bass_kernel_spmd(
            nc, in_maps, core_ids=list(range(NCORES)), trace=True)
        out = np.concatenate([r["out"] for r in res.results], axis=0)
        return out, res

    d = _get_disp(nb, _NBLK, has_bias)
    sh = d["sharding"]
    puts = {}
    with jax.default_device(cpu):
        # dispatch all host prep asynchronously (XLA-CPU queues them in order)
        chunks = [prep_wchunk(word_embs, i) for i in range(NWCHUNK)]
        rest = prep_rest(word_embs, object_embs, W, att_src, att_dst)
    # as each chunk's quantize completes, start its upload; the axon link
    # streams in the background while later chunks still compute
    for i, (q, sc) in enumerate(chunks):
        puts[f"xwq{i}"] = jax.device_put(np.asarray(q), sh)
        puts[f"xwsc{i}"] = jax.device_put(np.asarray(sc), sh)
    for k, v in zip(["xoq", "xosc", "wh", "swp", "sA2"], rest):
        puts[k] = jax.device_put(np.asarray(v), sh)
    if has_bias:
        puts["bias128"] = jax.device_put(np.tile(
            np.ascontiguousarray(bias.reshape(4, 128).T.astype(np.float32)),
            (NCORES, 1)), sh)
        puts["biasrow"] = jax.device_put(np.tile(
            bias.reshape(1, 512).astype(np.float16), (NCORES, 1)), sh)
    args = [puts[k] for k in d["in_names"]]
    zeros = [np.zeros((NCORES * s[0], *s[1:]), dt) for (s, dt) in d["out_shapes"]]
    out_arrs = d["sharded"](*args, *zeros, *d["extra"])
    out = np.asarray(out_arrs[0])
    return out, None


def kernel(**inputs) -> np.ndarray:
    return _run(inputs, trace=False)[0]


# revision 17
# speedup vs baseline: 1.1769x; 1.1457x over previous
"""Trainium2 Bass kernel for nn_ObjectWordGAT (8-core data parallel).

Self-contained: accepts FULL inputs, shards batch across 8 NeuronCores,
returns the FULL [256, 512] fp32 output.

Warm-path design (the wall clock is dominated by the ~73MB/s axon link and
~0.1s RPC latencies, not HW exec):
  - embeddings are uploaded as per-row int8 (natural row-major layout), and
    dequantized + transposed on-chip (scalar engine affine + PE transposes),
    eliminating both the host-side transpose and half the upload bytes;
  - attention scores s = x @ (W @ att) are computed exactly on host (tiny
    sgemm) and uploaded pre-packed (~0.6MB), removing the quantization error
    from the logit path;
  - wm (head-mean projection) is derived on-chip from wh;
  - the jitted shard_map dispatch closure is built once and cached, so warm
    calls skip retracing/recompiling;
  - all device_puts are issued asynchronously and overlap each other.
"""
import numpy as np
import concourse.mybir as mybir


# ======== gat_core.py ========

from contextlib import ExitStack

from concourse.masks import make_identity

F16 = mybir.dt.float16
F32 = mybir.dt.float32
I8 = mybir.dt.int8
AF = mybir.ActivationFunctionType
ALU = mybir.AluOpType
AX = mybir.AxisListType

D = 512
H = 2
E = 512
No = 36
Nw = 256
NEG = 0.2
NWCHUNK = 4  # word rows are uploaded in this many pipelined chunks


def build_gat(tc, out_ap, ins, nb=32, nblk=4, has_bias=False, dbg=None):
    def tap(name, ap):
        if dbg is not None and name in dbg:
            tc.nc.sync.dma_start(dbg[name][:], ap)

    nc = tc.nc
    xoq, xosc = ins["xoq"], ins["xosc"]
    wh, swp, sA2 = ins["wh"], ins["swp"], ins["sA2"]
    RW, RO = nb * Nw, nb * 64  # obj rows padded to 64 per b
    ROP = nb * No              # packed obj rows (36 per b)
    nbl = nb // nblk
    not_ = nb * Nw // 128      # word row tiles per core
    ntc = not_ // NWCHUNK      # word row tiles per upload chunk
    nto = ROP // 128           # obj row tiles per core (1152/128 = 9)
    assert ROP % 128 == 0
    assert nb % nblk == 0 and nblk % 2 == 0

    ctx = ExitStack()
    with ctx:
        const = ctx.enter_context(tc.tile_pool(name="const", bufs=1))
        # ---- constants ----
        wh_sb = [const.tile([128, 1024], F16, name=f"wh{c}", tag=f"wh{c}") for c in range(4)]
        wm_sb = [const.tile([128, 512], F16, name=f"wm{c}", tag=f"wm{c}") for c in range(4)]
        for c in range(4):
            sl = slice(c * 128, (c + 1) * 128)
            nc.sync.dma_start(wh_sb[c][:], wh[sl, :])
        ident16 = const.tile([128, 128], F16, name="id16", tag="id16")
        ident32 = const.tile([128, 128], F32, name="id32", tag="id32")
        make_identity(nc, ident16[:])
        make_identity(nc, ident32[:])
        ones16 = const.tile([1, 128], F16, name="ones16", tag="ones16")
        nc.vector.memset(ones16[:], 1.0)
        # wm = 0.5 * (wh_head0 + wh_head1), on-chip
        for c in range(4):
            nc.vector.tensor_add(wm_sb[c][:], wh_sb[c][:, 0:512], wh_sb[c][:, 512:1024])
            nc.scalar.mul(wm_sb[c][:], wm_sb[c][:], 0.5)
        # packed attention-score constants (computed on host, exact)
        swp_sb = const.tile([128, nbl * nblk * 8], F16, name="swp", tag="swp")
        nc.sync.dma_start(swp_sb[:], swp[:, :])
        sA2_sb = const.tile([1, nb * 148], F16, name="sA2", tag="sA2")
        nc.sync.dma_start(sA2_sb[:], sA2[:, :])
        # quant scales
        xwsc_sb = [const.tile([128, ntc], F32, name=f"xwsc{i}", tag=f"xwsc{i}")
                   for i in range(NWCHUNK)]
        for i in range(NWCHUNK):
            nc.sync.dma_start(xwsc_sb[i][:], ins[f"xwsc{i}"][:, :])
        xosc_sb = const.tile([128, nto], F32, name="xosc", tag="xosc")
        nc.sync.dma_start(xosc_sb[:], xosc[:, :])
        if has_bias:
            bias_sb = const.tile([128, 4], F32, name="bias128", tag="bias128")
            nc.sync.dma_start(bias_sb[:], ins["bias128"][:, :])
            biasrow_sb = const.tile([1, 512], F16, name="biasrow", tag="biasrow")
            nc.sync.dma_start(biasrow_sb[:], ins["biasrow"][:, :])

        # resident results
        ngrp2 = nb // 2  # obj rows padded: 2 b per 128-row tile
        xto_sb = [const.tile([128, RO], F16, name=f"xto{c}", tag=f"xto{c}") for c in range(4)]
        hobj_sb = const.tile([128, ngrp2 * 1024], F16, name="hobj", tag="hobj")
        uoT_sb = const.tile([128, 4 * RO], F16, name="uoT", tag="uoT")
        outT_sb = const.tile([128, nb * 4], F32, name="outT", tag="outT")

        # ================= PHASE 0: objects dequant + transpose =================
        # xoq [ROP, 512] int8 natural -> xto_sb[c] [128, RO] f16 (64-padded per b)
        with (
            tc.tile_pool(name="sb0", bufs=2) as sb0,
            tc.tile_pool(name="ps0", bufs=2, space="PSUM") as ps0,
        ):
            xtoP_sb = [const.tile([128, ROP], F16, name=f"xtoP{c}", tag=f"xtoP{c}")
                       for c in range(4)]
            for t in range(nto):
                xq_t = sb0.tile([128, 512], I8, name="xq", tag="xq")
                nc.sync.dma_start(xq_t[:], xoq[t * 128:(t + 1) * 128, :])
                xf_t = sb0.tile([128, 512], F16, name="xf", tag="xf")
                nc.scalar.mul(xf_t[:], xq_t[:], xosc_sb[:, t:t + 1])
                pt = ps0.tile([128, 512], F16, name="pt", tag="pt")
                for c in range(4):
                    nc.tensor.transpose(pt[:, c * 128:(c + 1) * 128],
                                        xf_t[:, c * 128:(c + 1) * 128], ident16[:])
                for c in range(4):
                    eng = nc.scalar.copy if (t + c) % 2 == 0 else nc.vector.tensor_copy
                    eng(xtoP_sb[c][:, t * 128:(t + 1) * 128],
                        pt[:, c * 128:(c + 1) * 128])
            # pad 36 -> 64 per b
            for c in range(4):
                nc.gpsimd.memset(
                    xto_sb[c][:].rearrange("p (b n) -> p b n", n=64)[:, :, No:64], 0.0)
                nc.vector.tensor_copy(
                    xto_sb[c][:].rearrange("p (b n) -> p b n", n=64)[:, :, 0:No],
                    xtoP_sb[c][:].rearrange("p (b n) -> p b n", n=No))
        tap("xto0", xto_sb[0][:])

        # ================= PHASE A: objects =================
        with tc.tile_pool(name="psA", bufs=2, space="PSUM") as psA:
            for g in range(ngrp2):
                pt = psA.tile([128, 1024], F32, name="phobj", tag="phobj")
                for he in range(2):
                    for c in range(4):
                        nc.tensor.matmul(
                            pt[:, he * 512:(he + 1) * 512],
                            lhsT=xto_sb[c][:, 128 * g:128 * (g + 1)],
                            rhs=wh_sb[c][:, he * 512:(he + 1) * 512],
                            start=(c == 0), stop=(c == 3),
                        )
                eng = nc.scalar.copy if g % 2 == 0 else nc.vector.tensor_copy
                eng(hobj_sb[:, g * 1024:(g + 1) * 1024], pt[:, :])

        with tc.tile_pool(name="psB", bufs=2, space="PSUM") as psB:
            # upd_obj^T = Wm.T @ Xo^T (+bias on evac)
            nchunks = [(i, min(512, RO - i)) for i in range(0, RO, 512)]
            for ec in range(4):
                for n0, nn in nchunks:
                    pt = psB.tile([128, 512], F32, name="puoT", tag="puoT")
                    for c in range(4):
                        nc.tensor.matmul(
                            pt[:, 0:nn],
                            lhsT=wm_sb[c][:, ec * 128:(ec + 1) * 128],
                            rhs=xto_sb[c][:, n0:n0 + nn],
                            start=(c == 0), stop=(c == 3),
                        )
                    dst = uoT_sb[:, ec * RO + n0: ec * RO + n0 + nn]
                    if has_bias:
                        nc.scalar.activation(dst, pt[:, 0:nn], AF.Identity,
                                             bias=bias_sb[:, ec:ec + 1])
                    elif (ec * len(nchunks) + n0 // 512) % 2 == 0:
                        nc.scalar.copy(dst, pt[:, 0:nn])
                    else:
                        nc.vector.tensor_copy(dst, pt[:, 0:nn])

        tap("hobj", hobj_sb[:])
        tap("uoT", uoT_sb[:])
        tap("sA2", sA2_sb[:])

        # ================= PHASE B: word blocks =================
        with (
            tc.tile_pool(name="sbB", bufs=2) as sbB,
            tc.tile_pool(name="ps_hw", bufs=2, space="PSUM") as ps_hw,
            tc.tile_pool(name="ps_mid", bufs=2, space="PSUM") as ps_mid,
            tc.tile_pool(name="ps_sm", bufs=2, space="PSUM") as ps_sm,
            tc.tile_pool(name="ps_aT", bufs=1, space="PSUM") as ps_aT,
        ):
            for blk in range(nbl):
                gw0 = blk * nblk * Nw  # first word row of block
                nwt = nblk * 2  # 128-row word tiles in block
                ng = nblk * 4   # (bi, whi, h) groups in block
                # ---- dequant + transpose words of this block ----
                xtw_sb = [sbB.tile([128, nblk * 256], F16, name=f"xtw{c}", tag=f"xtw{c}")
                          for c in range(4)]
                for wt8 in range(nwt):
                    t = blk * nwt + wt8
                    ch, tci = t // ntc, t % ntc
                    xq_t = sbB.tile([128, 512], I8, name="xqw", tag="xqw")
                    nc.sync.dma_start(
                        xq_t[:], ins[f"xwq{ch}"][tci * 128:(tci + 1) * 128, :])
                    xf_t = sbB.tile([128, 512], F16, name="xfw", tag="xfw")
                    nc.scalar.mul(xf_t[:], xq_t[:], xwsc_sb[ch][:, tci:tci + 1])
                    pt = ps_aT.tile([128, 512], F16, name="paT", tag="paT")
                    for c in range(4):
                        nc.tensor.transpose(pt[:, c * 128:(c + 1) * 128],
                                            xf_t[:, c * 128:(c + 1) * 128], ident16[:])
                    for c in range(4):
                        eng = nc.scalar.copy if (wt8 + c) % 2 == 0 else nc.vector.tensor_copy
                        eng(xtw_sb[c][:, wt8 * 128:(wt8 + 1) * 128],
                            pt[:, c * 128:(c + 1) * 128])

                # ---- s_word: packed slice of the host-computed scores ----
                sw_sb = swp_sb[:, blk * nwt * 4:(blk + 1) * nwt * 4]
                if blk == 0:
                    tap("sw", sw_sb)

                # ---- spread [128, nblk*148] = s_dst col per (bi,whi,h) ----
                spread_sb = sbB.tile([128, nblk * 148], F16, name="spread", tag="spread")
                src = sw_sb.rearrange("p (b whi f) -> p b whi f",
                                      b=nblk, whi=2)[:, :, :, 2:4]
                src = src.broadcast_to([128, nblk, 2, 2, 37])
                dst = spread_sb[:].rearrange("p (b whi h n) -> p b whi h n",
                                             b=nblk, whi=2, h=2)
                nc.vector.tensor_copy(dst, src)
                # self column (n=36): s_src + s_dst
                swg = sw_sb.rearrange("p (b whi f) -> p b whi f",
                                      b=nblk, whi=2)
                nc.vector.tensor_add(
                    dst[:, :, :, :, 36:37].rearrange("p b whi h n -> p b whi (h n)"),
                    dst[:, :, :, :, 36:37].rearrange("p b whi h n -> p b whi (h n)"),
                    swg[:, :, :, 0:2])

                # ---- L psums + lrelu + exp ----
                L2_sb = sbB.tile([128, nblk * 148], F32, name="L2", tag="L2")
                half = nblk * 148 // 2
                for hf in range(2):
                    p_L = ps_sm.tile([128, half], F32, name="sm", tag="sm")
                    nc.tensor.matmul(
                        p_L[:], lhsT=ones16[:],
                        rhs=sA2_sb[0:1, blk * nblk * 148 + hf * half:][:, 0:half],
                        start=True, stop=False)
                    nc.tensor.matmul(
                        p_L[:], lhsT=ident16[:],
                        rhs=spread_sb[:, hf * half:(hf + 1) * half],
                        start=False, stop=True)
                    ltmp = sbB.tile([128, half], F16, name="ltmp", tag="ltmp")
                    nc.scalar.mul(ltmp[:], p_L[:], NEG)
                    nc.vector.tensor_max(
                        L2_sb[:, hf * half:(hf + 1) * half], p_L[:], ltmp[:])
                expL_sb = sbB.tile([128, nblk * 148], F32, name="expL", tag="expL")
                nc.scalar.activation(expL_sb[:], L2_sb[:], AF.Exp)
                if blk == 0:
                    tap("L2", L2_sb[:])
                    tap("expL", expL_sb[:])

                # ---- den, r, alpha, c ----
                expg = expL_sb[:].rearrange("p (g n) -> p g n", n=37)
                den_sb = sbB.tile([128, ng], F32, name="den", tag="den")
                nc.vector.tensor_reduce(den_sb[:], expg, axis=AX.X, op=ALU.add)
                r_sb = sbB.tile([128, ng], F32, name="r", tag="r")
                nc.vector.reciprocal(r_sb[:], den_sb[:])
                nc.vector.tensor_scalar_mul(r_sb[:], r_sb[:], 0.5)
                alpha_sb = sbB.tile([128, ng * 64], F16, name="alpha", tag="alpha")
                nc.gpsimd.memset(
                    alpha_sb[:].rearrange("p (g n) -> p g n", n=64)[:, :, 36:64],
                    0.0)
                rbc = r_sb[:].broadcast_to([128, ng, 36])
                nc.vector.tensor_mul(
                    alpha_sb[:].rearrange("p (g n) -> p g n", n=64)[:, :, 0:36],
                    expg[:, :, 0:36], rbc)
                c_sb = sbB.tile([128, ng], F32, name="c", tag="c")
                nc.vector.tensor_mul(
                    c_sb[:],
                    expg[:, :, 36:37].rearrange("p g n -> p (g n)"), r_sb[:])
                if blk == 0:
                    tap("den", den_sb[:])
                    tap("alpha", alpha_sb[:])
                    tap("c", c_sb[:])

                # ---- alpha transposes -> aT [128, (nblk/2)*512] ----
                # partition half = b parity; col = pair*512 + h*256 + whi*128
                aT_sb = sbB.tile([128, (nblk // 2) * 512], F16, name="aT",
                                 tag="aT")
                for pr in range(nblk // 2):
                    p_aTt = ps_aT.tile([128, 512], F16, name="paT", tag="paT")
                    for pb in range(2):
                        bi = pr * 2 + pb
                        for whi in range(2):
                            for h in range(2):
                                g = (bi * 2 + whi) * 2 + h
                                nc.tensor.transpose(
                                    p_aTt[64 * pb:64 * pb + 64,
                                          h * 256 + whi * 128:][:, 0:128],
                                    alpha_sb[:, g * 64:(g + 1) * 64],
                                    ident16[:],
                                    tile_position=(0, 64 * pb),
                                )
                    nc.vector.tensor_copy(aT_sb[:, pr * 512:(pr + 1) * 512],
                                          p_aTt[:])

                if blk == 0:
                    tap("aT", aT_sb[:])
                # ---- h_word proj + t + msg + uw per (bi, whi) ----
                t_sb = sbB.tile([128, nwt * 512], F16, name="t", tag="t")
                uw_sb = sbB.tile([128, nwt * 512], F16, name="uw", tag="uw")
                for bi in range(nblk):
                    b = blk * nblk + bi
                    for whi in range(2):
                        wt = bi * 2 + whi
                        g = wt * 2  # (bi, whi, h=0)
                        p_he0 = ps_hw.tile([128, 512], F32, name="hw", tag="hw")
                        for c in range(4):
                            nc.tensor.matmul(
                                p_he0[:],
                                lhsT=xtw_sb[c][:, wt * 128:(wt + 1) * 128],
                                rhs=wh_sb[c][:, 0:512],
                                start=(c == 0), stop=(c == 3))
                        t0_sb = sbB.tile([128, 512], F16, name="t0", tag="t0")
                        nc.scalar.mul(t0_sb[:], p_he0[:], c_sb[:, g:g + 1])
                        p_he1 = ps_hw.tile([128, 512], F32, name="hw", tag="hw")
                        for c in range(4):
                            nc.tensor.matmul(
                                p_he1[:],
                                lhsT=xtw_sb[c][:, wt * 128:(wt + 1) * 128],
                                rhs=wh_sb[c][:, 512:1024],
                                start=(c == 0), stop=(c == 3))
                        t1_sb = sbB.tile([128, 512], F16, name="t1", tag="t1")
                        nc.vector.tensor_scalar_mul(t1_sb[:], p_he1[:],
                                                    c_sb[:, g + 1:g + 2])
                        nc.gpsimd.tensor_add(t_sb[:, wt * 512:(wt + 1) * 512],
                                             t0_sb[:], t1_sb[:])

                        # msg: two K=36 matmuls at row base 64*(b%2)
                        p_msg = ps_mid.tile([128, 512], F32, name="mid", tag="mid")
                        gq, go = b // 2, 64 * (b % 2)
                        acol = (bi // 2) * 512 + whi * 128
                        nc.tensor.matmul(
                            p_msg[:],
                            lhsT=aT_sb[go:go + 36, acol:acol + 128],
                            rhs=hobj_sb[go:go + 36, gq * 1024:gq * 1024 + 512],
                            start=True, stop=False,
                            tile_position=(go, 0))
                        nc.tensor.matmul(
                            p_msg[:],
                            lhsT=aT_sb[go:go + 36, acol + 256:acol + 256 + 128],
                            rhs=hobj_sb[go:go + 36,
                                        gq * 1024 + 512:gq * 1024 + 1024],
                            start=False, stop=not has_bias,
                            tile_position=(go, 0))
                        if has_bias:
                            nc.tensor.matmul(p_msg[:], lhsT=ones16[:],
                                             rhs=biasrow_sb[:],
                                             start=False, stop=True)
                        nc.vector.tensor_add(
                            uw_sb[:, wt * 512:(wt + 1) * 512], p_msg[:],
                            t_sb[:, wt * 512:(wt + 1) * 512])

                if blk == 0:
                    tap("t", t_sb[:])
                    tap("uw", uw_sb[:])
                # ---- uw transposes -> uwT [128, nblk*4*256] ----
                uwT_sb = sbB.tile([128, nblk * 4 * 256], F16, name="uwT", tag="uwT")
                for bi in range(nblk):
                    for ec in range(4):
                        p_uwT = ps_mid.tile([128, 256], F16, name="mid", tag="mid")
                        for whi in range(2):
                            nc.tensor.transpose(
                                p_uwT[:, whi * 128:(whi + 1) * 128],
                                uw_sb[:, (bi * 2 + whi) * 512 + ec * 128:][:, 0:128],
                                ident16[:])
                        dst = uwT_sb[:, (bi * 4 + ec) * 256:
                                     (bi * 4 + ec + 1) * 256]
                        if ec % 2 == 0:
                            nc.scalar.copy(dst, p_uwT[:])
                        else:
                            nc.vector.tensor_copy(dst, p_uwT[:])

                if blk == 0:
                    tap("uwT", uwT_sb[:])
                # ---- C + softmax + attnT ----
                p_attnT = ps_aT.tile([128, nblk * 2 * 36], F16, name="pattnT", tag="pattnT")
                for pair in range(nblk // 2):
                    p_C = ps_sm.tile([128, 256], F32, name="sm", tag="sm")
                    for pb in range(2):
                        bi = pair * 2 + pb
                        b = blk * nblk + bi
                        for ec in range(4):
                            nc.tensor.matmul(
                                p_C[64 * pb:64 * pb + 36, :],
                                lhsT=uoT_sb[:, ec * RO + b * 64:
                                            ec * RO + b * 64 + 36],
                                rhs=uwT_sb[:, (bi * 4 + ec) * 256:
                                           (bi * 4 + ec + 1) * 256],
                                start=(ec == 0), stop=(ec == 3),
                                tile_position=(0, 64 * pb))
                    negmax = sbB.tile([128, 1], F32, name="negmax", tag="negmax")
                    expC = sbB.tile([128, 256], F16, name="expC", tag="expC")
                    den2 = sbB.tile([128, 1], F32, name="den2", tag="den2")
                    rden = sbB.tile([128, 1], F32, name="rden", tag="rden")
                    attn = sbB.tile([128, 256], F16, name="attn", tag="attn")
                    for pb in range(2):
                        rs = slice(64 * pb, 64 * pb + 36)
                        nc.vector.tensor_reduce(negmax[rs], p_C[rs, :], axis=AX.X,
                                                op=ALU.max, negate=True)
                        nc.scalar.activation(expC[rs, :], p_C[rs, :], AF.Exp,
                                             bias=negmax[rs], accum_out=den2[rs])
                        nc.vector.reciprocal(rden[rs], den2[rs])
                        nc.vector.tensor_scalar_mul(rden[rs], rden[rs],
                                                    1.0 / 36.0)
                        nc.vector.tensor_scalar_mul(attn[rs, :], expC[rs, :],
                                                    rden[rs])
                    if blk == 0 and pair == 0:
                        tap("attn", attn[:])
                        tap("expC", expC[:])
                    for pb in range(2):
                        bi = pair * 2 + pb
                        for whi in range(2):
                            nc.tensor.transpose(
                                p_attnT[:, (bi * 2 + whi) * 36:
                                        (bi * 2 + whi + 1) * 36],
                                attn[64 * pb:64 * pb + 36,
                                     whi * 128:(whi + 1) * 128],
                                ident16[64 * pb:64 * pb + 36,
                                        64 * pb:64 * pb + 36],
                                tile_position=(64 * pb, 0))
                attnT_sb = sbB.tile([128, nblk * 2 * 36], F16, name="attnT", tag="attnT")
                nc.vector.tensor_copy(attnT_sb[:], p_attnT[:])
                if blk == 0:
                    tap("attnT", attnT_sb[:])

                # ---- weighted^T + final reduce ----
                for bi in range(nblk):
                    b = blk * nblk + bi
                    p_w = ps_sm.tile([128, 144], F32, name="sm", tag="sm")
                    for ec in range(4):
                        for whi in range(2):
                            nc.tensor.matmul(
                                p_w[:, ec * 36:(ec + 1) * 36],
                                lhsT=uw_sb[:, (bi * 2 + whi) * 512 +
                                           ec * 128:][:, 0:128],
                                rhs=attnT_sb[:, (bi * 2 + whi) * 36:
                                             (bi * 2 + whi + 1) * 36],
                                start=(whi == 0), stop=(whi == 1))
                    nc.vector.tensor_reduce(
                        outT_sb[:, b * 4:(b + 1) * 4],
                        p_w[:].rearrange("p (ec n) -> p ec n", n=36),
                        axis=AX.X, op=ALU.add)

        tap("outT", outT_sb[:])
        # ================= PHASE C: final transpose + store =================
        with tc.tile_pool(name="psC", bufs=1, space="PSUM") as psC:
            assert nb <= 128
            p_out = psC.tile([nb, 512], F32, name="p_out", tag="p_out")
            for ec in range(4):
                src = outT_sb[:].rearrange("p (b ec) -> p ec b", ec=4)[:, ec, :]
                nc.tensor.transpose(p_out[0:nb, ec * 128:(ec + 1) * 128],
                                    src, ident32[:])
            out_sb = const.tile([nb, 512], F32, name="out_sb", tag="out_sb")
            nc.vector.tensor_copy(out_sb[0:nb, :], p_out[0:nb, :])
            nc.sync.dma_start(out_ap[:, :], out_sb[0:nb, :])


# ======== runner.py ========

NCORES = 8
_B_TOTAL = 256
_NB = _B_TOTAL // NCORES  # 32
_NBLK = 4

_built = {}


def _build(nb, nblk, has_bias):
    key = (nb, nblk, has_bias)
    if key in _built:
        return _built[key]
    import concourse.bacc as bacc
    import concourse.tile as tile

    nc = bacc.Bacc(trn_type="TRN2", target_bir_lowering=False, debug=False,
                   num_devices=NCORES)
    f16 = mybir.dt.float16
    f32 = mybir.dt.float32
    i8 = mybir.dt.int8
    not_ = nb * Nw // 128
    ntc = not_ // NWCHUNK
    nto = nb * No // 128
    ins = {
        "xoq": nc.dram_tensor("xoq", [nb * No, 512], i8, kind="ExternalInput").ap(),
        "xosc": nc.dram_tensor("xosc", [128, nto], f32, kind="ExternalInput").ap(),
        "wh": nc.dram_tensor("wh", [512, 1024], f16, kind="ExternalInput").ap(),
        **{f"xwq{i}": nc.dram_tensor(f"xwq{i}", [ntc * 128, 512], i8,
                                     kind="ExternalInput").ap()
           for i in range(NWCHUNK)},
        **{f"xwsc{i}": nc.dram_tensor(f"xwsc{i}", [128, ntc], f32,
                                      kind="ExternalInput").ap()
           for i in range(NWCHUNK)},
        "swp": nc.dram_tensor("swp", [128, nb * 8], f16, kind="ExternalInput").ap(),
        "sA2": nc.dram_tensor("sA2", [1, nb * 148], f16, kind="ExternalInput").ap(),
    }
    if has_bias:
        ins["bias128"] = nc.dram_tensor("bias128", [128, 4], f32,
                                        kind="ExternalInput").ap()
        ins["biasrow"] = nc.dram_tensor("biasrow", [1, 512], f16,
                                        kind="ExternalInput").ap()
    out_ap = nc.dram_tensor("out", [nb, 512], f32, kind="ExternalOutput").ap()
    with tile.TileContext(nc) as tc:
        build_gat(tc, out_ap, ins, nb=nb, nblk=nblk, has_bias=has_bias)
    nc.compile()
    _built[key] = nc
    return nc


# ---- host-side packing (jax cpu jit, multithreaded) ----

_prep_jit = None


def _get_prep_jit():
    global _prep_jit
    if _prep_jit is not None:
        return _prep_jit
    import functools
    import jax
    import jax.numpy as jnp

    cpu = jax.devices("cpu")[0]

    @functools.partial(jax.jit, static_argnums=(1,))
    def _prep_wchunk(word_embs, i):
        # quantize word rows of upload chunk i: per-core rows
        # [i*rows_pc, (i+1)*rows_pc) with rows_pc = nb*Nw/NWCHUNK
        B = word_embs.shape[0]
        nb = B // NCORES
        rows_pc = nb * Nw // NWCHUNK
        ntc = rows_pc // 128
        wf = word_embs.reshape(NCORES, NWCHUNK, rows_pc, D)[:, i]
        wf = wf.reshape(NCORES * rows_pc, D)
        # per-row scale = 4.4x RMS of a 128-col sample (clipped below): the
        # sum-of-squares reduce vectorizes far better than an amax pass here
        wam = jnp.maximum(
            jnp.sqrt(jnp.mean(wf[:, :128] * wf[:, :128], axis=1)) * 4.4, 1e-20)
        ws = wam / 127.0
        q = jnp.clip(jnp.rint(wf * (1.0 / ws)[:, None]), -127, 127).astype(jnp.int8)
        sc = ws.reshape(NCORES, ntc, 128).transpose(0, 2, 1).reshape(
            NCORES * 128, ntc).astype(jnp.float32)
        return q, sc

    @jax.jit
    def _prep_rest(word_embs, object_embs, W, att_src, att_dst):
        B = word_embs.shape[0]
        nb = B // NCORES
        nbl = nb // _NBLK
        wf = word_embs.reshape(B * Nw, D)
        of = object_embs.reshape(B * No, D)
        oam = jnp.maximum(
            jnp.sqrt(jnp.mean(of[:, :128] * of[:, :128], axis=1)) * 4.4, 1e-20)
        osc = oam / 127.0
        xoq = jnp.clip(jnp.rint(of * (1.0 / osc)[:, None]), -127, 127).astype(jnp.int8)
        nto = nb * No // 128
        xosc = osc.reshape(NCORES, nto, 128).transpose(0, 2, 1).reshape(
            NCORES * 128, nto).astype(jnp.float32)
        # exact attention scores
        Wr = W.reshape(D, H, E)
        wa_src = jnp.einsum('dhe,he->dh', Wr, att_src)
        wa_dst = jnp.einsum('dhe,he->dh', Wr, att_dst)
        waf = jnp.concatenate([wa_src, wa_dst], axis=1)  # [D, 4]
        s_w = wf @ waf   # [B*Nw, 4]
        s_o = of @ wa_src  # [B*No, 2]
        # swp [core*128, nb*8]: col = blk*nwt*4 + wt*4 + f ; row ~ (core, p)
        nwt = _NBLK * 2
        swp = s_w.reshape(NCORES, nbl, nwt, 128, 4).transpose(0, 3, 1, 2, 4)
        swp = swp.reshape(NCORES * 128, nb * 8).astype(jnp.float16)
        # sA2 [core, nb*148]: col = b*148 + whi*74 + h*37 + n
        so = s_o.reshape(NCORES, nb, No, H).transpose(0, 1, 3, 2)  # [c, b, h, n]
        sA2 = jnp.zeros((NCORES, nb, 2, H, 37), jnp.float16)
        sA2 = sA2.at[:, :, :, :, 0:No].set(
            so[:, :, None, :, :].astype(jnp.float16))
        sA2 = sA2.reshape(NCORES, nb * 148)
        # wh replicated
        wh = jnp.tile(W.astype(jnp.float16), (NCORES, 1))
        return xoq, xosc, wh, swp, sA2

    _prep_jit = (_prep_wchunk, _prep_rest, cpu)
    return _prep_jit


# ---- cached PJRT dispatch (one jit closure per build, reused warm) ----

_disp = {}


def _get_disp(nb, nblk, has_bias):
    key = (nb, nblk, has_bias)
    if key in _disp:
        return _disp[key]
    import jax
    from jax.sharding import Mesh, PartitionSpec, NamedSharding
    from jax.experimental.shard_map import shard_map
    from concourse import bass2jax

    nc = _build(nb, nblk, has_bias)
    bass2jax.install_neuronx_cc_hook()
    assert nc.dbg_addr is None or not nc.dbg_callbacks
    partition_name = nc.partition_id_tensor.name if nc.partition_id_tensor else None

    in_names, out_names, out_avals, out_shapes = [], [], [], []
    for alloc in nc.m.functions[0].allocations:
        if not isinstance(alloc, mybir.MemoryLocationSet):
            continue
        name = alloc.memorylocations[0].name
        if alloc.kind == "ExternalInput":
            if name != partition_name:
                in_names.append(name)
        elif alloc.kind == "ExternalOutput":
            shape = tuple(alloc.tensor_shape)
            dtype = mybir.dt.np(alloc.dtype)
            out_names.append(name)
            out_avals.append(jax.core.ShapedArray(shape, dtype))
            out_shapes.append((shape, dtype))
    n_params = len(in_names)
    n_outs = len(out_avals)
    in_names_all = list(in_names) + list(out_names)
    if partition_name is not None:
        in_names_all.append(partition_name)
    extra = []
    if nc.dbg_addr is not None:
        in_names_all.append(nc.dbg_addr.name)
        extra.append(np.zeros((1, 2), np.uint32))

    donate = tuple(range(n_params, n_params + n_outs))

    def _body(*args):
        operands = list(args)
        if partition_name is not None:
            operands.append(bass2jax.partition_id_tensor())
        outs = bass2jax._bass_exec_p.bind(
            *operands,
            out_avals=tuple(out_avals),
            in_names=tuple(in_names_all),
            out_names=tuple(out_names),
            lowering_input_output_aliases=(),
            sim_require_finite=True,
            sim_require_nnan=True,
            nc=nc,
        )
        return tuple(outs)

    devices = jax.devices()[:NCORES]
    mesh = Mesh(np.asarray(devices), ("core",))
    nargs = n_params + n_outs + len(extra)
    in_specs = (PartitionSpec("core"),) * nargs
    out_specs = (PartitionSpec("core"),) * n_outs
    sharded = jax.jit(
        shard_map(_body, mesh=mesh, in_specs=in_specs, out_specs=out_specs,
                  check_rep=False),
        donate_argnums=donate, keep_unused=True,
    )
    sh = NamedSharding(mesh, PartitionSpec("core"))
    d = {
        "sharded": sharded, "sharding": sh, "in_names": in_names,
        "out_shapes": out_shapes, "extra": extra,
    }
    _disp[key] = d
    return d


def _run(inputs, trace=False):
    import jax

    object_embs = np.asarray(inputs["object_embs"], np.float32)
    word_embs = np.asarray(inputs["word_embs"], np.float32)
    W = np.asarray(inputs["W"], np.float32)
    att_src = np.asarray(inputs["att_src"], np.float32)
    att_dst = np.asarray(inputs["att_dst"], np.float32)
    bias = np.asarray(inputs["bias"], np.float32)
    has_bias = bool(np.any(bias))
    B = object_embs.shape[0]
    nb = B // NCORES

    (prep_wchunk, prep_rest, cpu) = _get_prep_jit()
    if trace:
        # profiling path: per-core in_maps through run_bass_kernel_spmd
        with jax.default_device(cpu):
            chunks = [prep_wchunk(word_embs, i) for i in range(NWCHUNK)]
            rest = prep_rest(word_embs, object_embs, W, att_src, att_dst)
        host = {}
        for i, (q, sc) in enumerate(chunks):
            host[f"xwq{i}"] = np.asarray(q)
            host[f"xwsc{i}"] = np.asarray(sc)
        for k, v in zip(["xoq", "xosc", "wh", "swp", "sA2"], rest):
            host[k] = np.asarray(v)
        if has_bias:
            host["bias128"] = np.tile(
                np.ascontiguousarray(bias.reshape(4, 128).T.astype(np.float32)),
                (NCORES, 1))
            host["biasrow"] = np.tile(
                bias.reshape(1, 512).astype(np.float16), (NCORES, 1))
        from concourse import bass_utils
        nc = _build(nb, _NBLK, has_bias)
        in_maps = []
        for core in range(NCORES):
            m = {}
            for k, v in host.items():
                rows = v.shape[0] // NCORES
                m[k] = np.ascontiguousarray(v[core * rows:(core + 1) * rows])
            in_maps.append(m)
        res = bass_utils.run_bass_kernel_spmd(
            nc, in_maps, core_ids=list(range(NCORES)), trace=True)
        out = np.concatenate([r["out"] for r in res.results], axis=0)
        return out, res

    d = _get_disp(nb, _NBLK, has_bias)
    sh = d["sharding"]
    puts = {}
    with jax.default_device(cpu):
        # dispatch all host prep asynchronously (XLA-CPU queues them in order)
        chunks = [prep_wchunk(word_embs, i) for i in range(NWCHUNK)]
        rest = prep_rest(word_embs, object_embs, W, att_src, att_dst)
    # as each chunk's quantize completes, start its upload; the axon link
    # streams in the background while later chunks still compute
    for i, (q, sc) in enumerate(chunks):
        a, b = jax.device_put([np.asarray(q), np.asarray(sc)], [sh, sh])
        puts[f"xwq{i}"] = a
        puts[f"xwsc{i}"] = b
    ks = ["xoq", "xosc", "wh", "swp", "sA2"]
    vals = [np.asarray(v) for v in rest]
    puts.update(dict(zip(ks, jax.device_put(vals, [sh] * len(vals)))))
    if has_bias:
        puts["bias128"] = jax.device_put(np.tile(
            np.ascontiguousarray(bias.reshape(4, 128).T.astype(np.float32)),
            (NCORES, 1)), sh)
        puts["biasrow"] = jax.device_put(np.tile(
            bias.reshape(1, 512).astype(np.float16), (NCORES, 1)), sh)
    args = [puts[k] for k in d["in_names"]]
    zeros = [np.zeros((NCORES * s[0], *s[1:]), dt) for (s, dt) in d["out_shapes"]]
    out_arrs = d["sharded"](*args, *zeros, *d["extra"])
    out = np.asarray(out_arrs[0])
    return out, None


def kernel(**inputs) -> np.ndarray:
    return _run(inputs, trace=False)[0]


# revision 18
# speedup vs baseline: 1.1785x; 1.0013x over previous
"""Trainium2 Bass kernel for nn_ObjectWordGAT (8-core data parallel).

Self-contained: accepts FULL inputs, shards batch across 8 NeuronCores,
returns the FULL [256, 512] fp32 output.

Warm-path design (the wall clock is dominated by the ~73MB/s axon link and
~0.1s RPC latencies, not HW exec):
  - embeddings are uploaded as per-row int8 (natural row-major layout), and
    dequantized + transposed on-chip (scalar engine affine + PE transposes),
    eliminating both the host-side transpose and half the upload bytes;
  - attention scores s = x @ (W @ att) are computed exactly on host (tiny
    sgemm) and uploaded pre-packed (~0.6MB), removing the quantization error
    from the logit path;
  - wm (head-mean projection) is derived on-chip from wh;
  - the jitted shard_map dispatch closure is built once and cached, so warm
    calls skip retracing/recompiling;
  - all device_puts are issued asynchronously and overlap each other.
"""
import numpy as np
import concourse.mybir as mybir


# ======== gat_core.py ========

from contextlib import ExitStack

from concourse.masks import make_identity

F16 = mybir.dt.float16
F32 = mybir.dt.float32
I8 = mybir.dt.int8
AF = mybir.ActivationFunctionType
ALU = mybir.AluOpType
AX = mybir.AxisListType

D = 512
H = 2
E = 512
No = 36
Nw = 256
NEG = 0.2
NWCHUNK = 4  # word rows are uploaded in this many pipelined chunks


def build_gat(tc, out_ap, ins, nb=32, nblk=4, has_bias=False, dbg=None):
    def tap(name, ap):
        if dbg is not None and name in dbg:
            tc.nc.sync.dma_start(dbg[name][:], ap)

    nc = tc.nc
    xoq, xosc = ins["xoq"], ins["xosc"]
    wh, swp, sA2 = ins["wh"], ins["swp"], ins["sA2"]
    RW, RO = nb * Nw, nb * 64  # obj rows padded to 64 per b
    ROP = nb * No              # packed obj rows (36 per b)
    nbl = nb // nblk
    not_ = nb * Nw // 128      # word row tiles per core
    ntc = not_ // NWCHUNK      # word row tiles per upload chunk
    nto = ROP // 128           # obj row tiles per core (1152/128 = 9)
    assert ROP % 128 == 0
    assert nb % nblk == 0 and nblk % 2 == 0

    ctx = ExitStack()
    with ctx:
        const = ctx.enter_context(tc.tile_pool(name="const", bufs=1))
        # ---- constants ----
        wh_sb = [const.tile([128, 1024], F16, name=f"wh{c}", tag=f"wh{c}") for c in range(4)]
        wm_sb = [const.tile([128, 512], F16, name=f"wm{c}", tag=f"wm{c}") for c in range(4)]
        for c in range(4):
            sl = slice(c * 128, (c + 1) * 128)
            nc.sync.dma_start(wh_sb[c][:], wh[sl, :])
        ident16 = const.tile([128, 128], F16, name="id16", tag="id16")
        ident32 = const.tile([128, 128], F32, name="id32", tag="id32")
        make_identity(nc, ident16[:])
        make_identity(nc, ident32[:])
        ones16 = const.tile([1, 128], F16, name="ones16", tag="ones16")
        nc.vector.memset(ones16[:], 1.0)
        # wm = 0.5 * (wh_head0 + wh_head1), on-chip
        for c in range(4):
            nc.vector.tensor_add(wm_sb[c][:], wh_sb[c][:, 0:512], wh_sb[c][:, 512:1024])
            nc.scalar.mul(wm_sb[c][:], wm_sb[c][:], 0.5)
        # packed attention-score constants (computed on host, exact)
        swp_sb = const.tile([128, nbl * nblk * 8], F16, name="swp", tag="swp")
        nc.sync.dma_start(swp_sb[:], swp[:, :])
        sA2_sb = const.tile([1, nb * 148], F16, name="sA2", tag="sA2")
        nc.sync.dma_start(sA2_sb[:], sA2[:, :])
        # quant scales
        xwsc_sb = [const.tile([128, ntc], F32, name=f"xwsc{i}", tag=f"xwsc{i}")
                   for i in range(NWCHUNK)]
        for i in range(NWCHUNK):
            nc.sync.dma_start(xwsc_sb[i][:], ins[f"xwsc{i}"][:, :])
        xosc_sb = const.tile([128, nto], F32, name="xosc", tag="xosc")
        nc.sync.dma_start(xosc_sb[:], xosc[:, :])
        if has_bias:
            bias_sb = const.tile([128, 4], F32, name="bias128", tag="bias128")
            nc.sync.dma_start(bias_sb[:], ins["bias128"][:, :])
            biasrow_sb = const.tile([1, 512], F16, name="biasrow", tag="biasrow")
            nc.sync.dma_start(biasrow_sb[:], ins["biasrow"][:, :])

        # resident results
        ngrp2 = nb // 2  # obj rows padded: 2 b per 128-row tile
        xto_sb = [const.tile([128, RO], F16, name=f"xto{c}", tag=f"xto{c}") for c in range(4)]
        hobj_sb = const.tile([128, ngrp2 * 1024], F16, name="hobj", tag="hobj")
        uoT_sb = const.tile([128, 4 * RO], F16, name="uoT", tag="uoT")
        outT_sb = const.tile([128, nb * 4], F32, name="outT", tag="outT")

        # ================= PHASE 0: objects dequant + transpose =================
        # xoq [ROP, 512] int8 natural -> xto_sb[c] [128, RO] f16 (64-padded per b)
        with (
            tc.tile_pool(name="sb0", bufs=2) as sb0,
            tc.tile_pool(name="ps0", bufs=2, space="PSUM") as ps0,
        ):
            xtoP_sb = [const.tile([128, ROP], F16, name=f"xtoP{c}", tag=f"xtoP{c}")
                       for c in range(4)]
            for t in range(nto):
                xq_t = sb0.tile([128, 512], I8, name="xq", tag="xq")
                nc.sync.dma_start(xq_t[:], xoq[t * 128:(t + 1) * 128, :])
                xf_t = sb0.tile([128, 512], F16, name="xf", tag="xf")
                nc.scalar.mul(xf_t[:], xq_t[:], xosc_sb[:, t:t + 1])
                pt = ps0.tile([128, 512], F16, name="pt", tag="pt")
                for c in range(4):
                    nc.tensor.transpose(pt[:, c * 128:(c + 1) * 128],
                                        xf_t[:, c * 128:(c + 1) * 128], ident16[:])
                for c in range(4):
                    eng = nc.scalar.copy if (t + c) % 2 == 0 else nc.vector.tensor_copy
                    eng(xtoP_sb[c][:, t * 128:(t + 1) * 128],
                        pt[:, c * 128:(c + 1) * 128])
            # pad 36 -> 64 per b
            for c in range(4):
                nc.gpsimd.memset(
                    xto_sb[c][:].rearrange("p (b n) -> p b n", n=64)[:, :, No:64], 0.0)
                nc.vector.tensor_copy(
                    xto_sb[c][:].rearrange("p (b n) -> p b n", n=64)[:, :, 0:No],
                    xtoP_sb[c][:].rearrange("p (b n) -> p b n", n=No))
        tap("xto0", xto_sb[0][:])

        # ================= PHASE A: objects =================
        with tc.tile_pool(name="psA", bufs=2, space="PSUM") as psA:
            for g in range(ngrp2):
                pt = psA.tile([128, 1024], F32, name="phobj", tag="phobj")
                for he in range(2):
                    for c in range(4):
                        nc.tensor.matmul(
                            pt[:, he * 512:(he + 1) * 512],
                            lhsT=xto_sb[c][:, 128 * g:128 * (g + 1)],
                            rhs=wh_sb[c][:, he * 512:(he + 1) * 512],
                            start=(c == 0), stop=(c == 3),
                        )
                eng = nc.scalar.copy if g % 2 == 0 else nc.vector.tensor_copy
                eng(hobj_sb[:, g * 1024:(g + 1) * 1024], pt[:, :])

        with tc.tile_pool(name="psB", bufs=2, space="PSUM") as psB:
            # upd_obj^T = Wm.T @ Xo^T (+bias on evac)
            nchunks = [(i, min(512, RO - i)) for i in range(0, RO, 512)]
            for ec in range(4):
                for n0, nn in nchunks:
                    pt = psB.tile([128, 512], F32, name="puoT", tag="puoT")
                    for c in range(4):
                        nc.tensor.matmul(
                            pt[:, 0:nn],
                            lhsT=wm_sb[c][:, ec * 128:(ec + 1) * 128],
                            rhs=xto_sb[c][:, n0:n0 + nn],
                            start=(c == 0), stop=(c == 3),
                        )
                    dst = uoT_sb[:, ec * RO + n0: ec * RO + n0 + nn]
                    if has_bias:
                        nc.scalar.activation(dst, pt[:, 0:nn], AF.Identity,
                                             bias=bias_sb[:, ec:ec + 1])
                    elif (ec * len(nchunks) + n0 // 512) % 2 == 0:
                        nc.scalar.copy(dst, pt[:, 0:nn])
                    else:
                        nc.vector.tensor_copy(dst, pt[:, 0:nn])

        tap("hobj", hobj_sb[:])
        tap("uoT", uoT_sb[:])
        tap("sA2", sA2_sb[:])

        # ================= PHASE B: word blocks =================
        with (
            tc.tile_pool(name="sbB", bufs=2) as sbB,
            tc.tile_pool(name="ps_hw", bufs=2, space="PSUM") as ps_hw,
            tc.tile_pool(name="ps_mid", bufs=2, space="PSUM") as ps_mid,
            tc.tile_pool(name="ps_sm", bufs=2, space="PSUM") as ps_sm,
            tc.tile_pool(name="ps_aT", bufs=1, space="PSUM") as ps_aT,
        ):
            for blk in range(nbl):
                gw0 = blk * nblk * Nw  # first word row of block
                nwt = nblk * 2  # 128-row word tiles in block
                ng = nblk * 4   # (bi, whi, h) groups in block
                # ---- dequant + transpose words of this block ----
                xtw_sb = [sbB.tile([128, nblk * 256], F16, name=f"xtw{c}", tag=f"xtw{c}")
                          for c in range(4)]
                for wt8 in range(nwt):
                    t = blk * nwt + wt8
                    ch, tci = t // ntc, t % ntc
                    xq_t = sbB.tile([128, 512], I8, name="xqw", tag="xqw")
                    nc.sync.dma_start(
                        xq_t[:], ins[f"xwq{ch}"][tci * 128:(tci + 1) * 128, :])
                    xf_t = sbB.tile([128, 512], F16, name="xfw", tag="xfw")
                    nc.scalar.mul(xf_t[:], xq_t[:], xwsc_sb[ch][:, tci:tci + 1])
                    pt = ps_aT.tile([128, 512], F16, name="paT", tag="paT")
                    for c in range(4):
                        nc.tensor.transpose(pt[:, c * 128:(c + 1) * 128],
                                            xf_t[:, c * 128:(c + 1) * 128], ident16[:])
                    for c in range(4):
                        eng = nc.scalar.copy if (wt8 + c) % 2 == 0 else nc.vector.tensor_copy
                        eng(xtw_sb[c][:, wt8 * 128:(wt8 + 1) * 128],
                            pt[:, c * 128:(c + 1) * 128])

                # ---- s_word: packed slice of the host-computed scores ----
                sw_sb = swp_sb[:, blk * nwt * 4:(blk + 1) * nwt * 4]
                if blk == 0:
                    tap("sw", sw_sb)

                # ---- spread [128, nblk*148] = s_dst col per (bi,whi,h) ----
                spread_sb = sbB.tile([128, nblk * 148], F16, name="spread", tag="spread")
                src = sw_sb.rearrange("p (b whi f) -> p b whi f",
                                      b=nblk, whi=2)[:, :, :, 2:4]
                src = src.broadcast_to([128, nblk, 2, 2, 37])
                dst = spread_sb[:].rearrange("p (b whi h n) -> p b whi h n",
                                             b=nblk, whi=2, h=2)
                nc.vector.tensor_copy(dst, src)
                # self column (n=36): s_src + s_dst
                swg = sw_sb.rearrange("p (b whi f) -> p b whi f",
                                      b=nblk, whi=2)
                nc.vector.tensor_add(
                    dst[:, :, :, :, 36:37].rearrange("p b whi h n -> p b whi (h n)"),
                    dst[:, :, :, :, 36:37].rearrange("p b whi h n -> p b whi (h n)"),
                    swg[:, :, :, 0:2])

                # ---- L psums + lrelu + exp ----
                L2_sb = sbB.tile([128, nblk * 148], F32, name="L2", tag="L2")
                half = nblk * 148 // 2
                for hf in range(2):
                    p_L = ps_sm.tile([128, half], F32, name="sm", tag="sm")
                    nc.tensor.matmul(
                        p_L[:], lhsT=ones16[:],
                        rhs=sA2_sb[0:1, blk * nblk * 148 + hf * half:][:, 0:half],
                        start=True, stop=False)
                    nc.tensor.matmul(
                        p_L[:], lhsT=ident16[:],
                        rhs=spread_sb[:, hf * half:(hf + 1) * half],
                        start=False, stop=True)
                    ltmp = sbB.tile([128, half], F16, name="ltmp", tag="ltmp")
                    nc.scalar.mul(ltmp[:], p_L[:], NEG)
                    nc.vector.tensor_max(
                        L2_sb[:, hf * half:(hf + 1) * half], p_L[:], ltmp[:])
                expL_sb = sbB.tile([128, nblk * 148], F32, name="expL", tag="expL")
                nc.scalar.activation(expL_sb[:], L2_sb[:], AF.Exp)
                if blk == 0:
                    tap("L2", L2_sb[:])
                    tap("expL", expL_sb[:])

                # ---- den, r, alpha, c ----
                expg = expL_sb[:].rearrange("p (g n) -> p g n", n=37)
                den_sb = sbB.tile([128, ng], F32, name="den", tag="den")
                nc.vector.tensor_reduce(den_sb[:], expg, axis=AX.X, op=ALU.add)
                r_sb = sbB.tile([128, ng], F32, name="r", tag="r")
                nc.vector.reciprocal(r_sb[:], den_sb[:])
                nc.vector.tensor_scalar_mul(r_sb[:], r_sb[:], 0.5)
                alpha_sb = sbB.tile([128, ng * 64], F16, name="alpha", tag="alpha")
                nc.gpsimd.memset(
                    alpha_sb[:].rearrange("p (g n) -> p g n", n=64)[:, :, 36:64],
                    0.0)
                rbc = r_sb[:].broadcast_to([128, ng, 36])
                nc.vector.tensor_mul(
                    alpha_sb[:].rearrange("p (g n) -> p g n", n=64)[:, :, 0:36],
                    expg[:, :, 0:36], rbc)
                c_sb = sbB.tile([128, ng], F32, name="c", tag="c")
                nc.vector.tensor_mul(
                    c_sb[:],
                    expg[:, :, 36:37].rearrange("p g n -> p (g n)"), r_sb[:])
                if blk == 0:
                    tap("den", den_sb[:])
                    tap("alpha", alpha_sb[:])
                    tap("c", c_sb[:])

                # ---- alpha transposes -> aT [128, (nblk/2)*512] ----
                # partition half = b parity; col = pair*512 + h*256 + whi*128
                aT_sb = sbB.tile([128, (nblk // 2) * 512], F16, name="aT",
                                 tag="aT")
                for pr in range(nblk // 2):
                    p_aTt = ps_aT.tile([128, 512], F16, name="paT", tag="paT")
                    for pb in range(2):
                        bi = pr * 2 + pb
                        for whi in range(2):
                            for h in range(2):
                                g = (bi * 2 + whi) * 2 + h
                                nc.tensor.transpose(
                                    p_aTt[64 * pb:64 * pb + 64,
                                          h * 256 + whi * 128:][:, 0:128],
                                    alpha_sb[:, g * 64:(g + 1) * 64],
                                    ident16[:],
                                    tile_position=(0, 64 * pb),
                                )
                    nc.vector.tensor_copy(aT_sb[:, pr * 512:(pr + 1) * 512],
                                          p_aTt[:])

                if blk == 0:
                    tap("aT", aT_sb[:])
                # ---- h_word proj + t + msg + uw per (bi, whi) ----
                t_sb = sbB.tile([128, nwt * 512], F16, name="t", tag="t")
                uw_sb = sbB.tile([128, nwt * 512], F16, name="uw", tag="uw")
                for bi in range(nblk):
                    b = blk * nblk + bi
                    for whi in range(2):
                        wt = bi * 2 + whi
                        g = wt * 2  # (bi, whi, h=0)
                        p_he0 = ps_hw.tile([128, 512], F32, name="hw", tag="hw")
                        for c in range(4):
                            nc.tensor.matmul(
                                p_he0[:],
                                lhsT=xtw_sb[c][:, wt * 128:(wt + 1) * 128],
                                rhs=wh_sb[c][:, 0:512],
                                start=(c == 0), stop=(c == 3))
                        t0_sb = sbB.tile([128, 512], F16, name="t0", tag="t0")
                        nc.scalar.mul(t0_sb[:], p_he0[:], c_sb[:, g:g + 1])
                        p_he1 = ps_hw.tile([128, 512], F32, name="hw", tag="hw")
                        for c in range(4):
                            nc.tensor.matmul(
                                p_he1[:],
                                lhsT=xtw_sb[c][:, wt * 128:(wt + 1) * 128],
                                rhs=wh_sb[c][:, 512:1024],
                                start=(c == 0), stop=(c == 3))
                        t1_sb = sbB.tile([128, 512], F16, name="t1", tag="t1")
                        nc.vector.tensor_scalar_mul(t1_sb[:], p_he1[:],
                                                    c_sb[:, g + 1:g + 2])
                        nc.gpsimd.tensor_add(t_sb[:, wt * 512:(wt + 1) * 512],
                                             t0_sb[:], t1_sb[:])

                        # msg: two K=36 matmuls at row base 64*(b%2)
                        p_msg = ps_mid.tile([128, 512], F32, name="mid", tag="mid")
                        gq, go = b // 2, 64 * (b % 2)
                        acol = (bi // 2) * 512 + whi * 128
                        nc.tensor.matmul(
                            p_msg[:],
                            lhsT=aT_sb[go:go + 36, acol:acol + 128],
                            rhs=hobj_sb[go:go + 36, gq * 1024:gq * 1024 + 512],
                            start=True, stop=False,
                            tile_position=(go, 0))
                        nc.tensor.matmul(
                            p_msg[:],
                            lhsT=aT_sb[go:go + 36, acol + 256:acol + 256 + 128],
                            rhs=hobj_sb[go:go + 36,
                                        gq * 1024 + 512:gq * 1024 + 1024],
                            start=False, stop=not has_bias,
                            tile_position=(go, 0))
                        if has_bias:
                            nc.tensor.matmul(p_msg[:], lhsT=ones16[:],
                                             rhs=biasrow_sb[:],
                                             start=False, stop=True)
                        nc.vector.tensor_add(
                            uw_sb[:, wt * 512:(wt + 1) * 512], p_msg[:],
                            t_sb[:, wt * 512:(wt + 1) * 512])

                if blk == 0:
                    tap("t", t_sb[:])
                    tap("uw", uw_sb[:])
                # ---- uw transposes -> uwT [128, nblk*4*256] ----
                uwT_sb = sbB.tile([128, nblk * 4 * 256], F16, name="uwT", tag="uwT")
                for bi in range(nblk):
                    for ec in range(4):
                        p_uwT = ps_mid.tile([128, 256], F16, name="mid", tag="mid")
                        for whi in range(2):
                            nc.tensor.transpose(
                                p_uwT[:, whi * 128:(whi + 1) * 128],
                                uw_sb[:, (bi * 2 + whi) * 512 + ec * 128:][:, 0:128],
                                ident16[:])
                        dst = uwT_sb[:, (bi * 4 + ec) * 256:
                                     (bi * 4 + ec + 1) * 256]
                        if ec % 2 == 0:
                            nc.scalar.copy(dst, p_uwT[:])
                        else:
                            nc.vector.tensor_copy(dst, p_uwT[:])

                if blk == 0:
                    tap("uwT", uwT_sb[:])
                # ---- C + softmax + attnT ----
                p_attnT = ps_aT.tile([128, nblk * 2 * 36], F16, name="pattnT", tag="pattnT")
                for pair in range(nblk // 2):
                    p_C = ps_sm.tile([128, 256], F32, name="sm", tag="sm")
                    for pb in range(2):
                        bi = pair * 2 + pb
                        b = blk * nblk + bi
                        for ec in range(4):
                            nc.tensor.matmul(
                                p_C[64 * pb:64 * pb + 36, :],
                                lhsT=uoT_sb[:, ec * RO + b * 64:
                                            ec * RO + b * 64 + 36],
                                rhs=uwT_sb[:, (bi * 4 + ec) * 256:
                                           (bi * 4 + ec + 1) * 256],
                                start=(ec == 0), stop=(ec == 3),
                                tile_position=(0, 64 * pb))
                    negmax = sbB.tile([128, 1], F32, name="negmax", tag="negmax")
                    expC = sbB.tile([128, 256], F16, name="expC", tag="expC")
                    den2 = sbB.tile([128, 1], F32, name="den2", tag="den2")
                    rden = sbB.tile([128, 1], F32, name="rden", tag="rden")
                    attn = sbB.tile([128, 256], F16, name="attn", tag="attn")
                    for pb in range(2):
                        rs = slice(64 * pb, 64 * pb + 36)
                        nc.vector.tensor_reduce(negmax[rs], p_C[rs, :], axis=AX.X,
                                                op=ALU.max, negate=True)
                        nc.scalar.activation(expC[rs, :], p_C[rs, :], AF.Exp,
                                             bias=negmax[rs], accum_out=den2[rs])
                        nc.vector.reciprocal(rden[rs], den2[rs])
                        nc.vector.tensor_scalar_mul(rden[rs], rden[rs],
                                                    1.0 / 36.0)
                        nc.vector.tensor_scalar_mul(attn[rs, :], expC[rs, :],
                                                    rden[rs])
                    if blk == 0 and pair == 0:
                        tap("attn", attn[:])
                        tap("expC", expC[:])
                    for pb in range(2):
                        bi = pair * 2 + pb
                        for whi in range(2):
                            nc.tensor.transpose(
                                p_attnT[:, (bi * 2 + whi) * 36:
                                        (bi * 2 + whi + 1) * 36],
                                attn[64 * pb:64 * pb + 36,
                                     whi * 128:(whi + 1) * 128],
                                ident16[64 * pb:64 * pb + 36,
                                        64 * pb:64 * pb + 36],
                                tile_position=(64 * pb, 0))
                attnT_sb = sbB.tile([128, nblk * 2 * 36], F16, name="attnT", tag="attnT")
                nc.vector.tensor_copy(attnT_sb[:], p_attnT[:])
                if blk == 0:
                    tap("attnT", attnT_sb[:])

                # ---- weighted^T + final reduce ----
                for bi in range(nblk):
                    b = blk * nblk + bi
                    p_w = ps_sm.tile([128, 144], F32, name="sm", tag="sm")
                    for ec in range(4):
                        for whi in range(2):
                            nc.tensor.matmul(
                                p_w[:, ec * 36:(ec + 1) * 36],
                                lhsT=uw_sb[:, (bi * 2 + whi) * 512 +
                                           ec * 128:][:, 0:128],
                                rhs=attnT_sb[:, (bi * 2 + whi) * 36:
                                             (bi * 2 + whi + 1) * 36],
                                start=(whi == 0), stop=(whi == 1))
                    nc.vector.tensor_reduce(
                        outT_sb[:, b * 4:(b + 1) * 4],
                        p_w[:].rearrange("p (ec n) -> p ec n", n=36),
                        axis=AX.X, op=ALU.add)

        tap("outT", outT_sb[:])
        # ================= PHASE C: final transpose + store =================
        with tc.tile_pool(name="psC", bufs=1, space="PSUM") as psC:
            assert nb <= 128
            p_out = psC.tile([nb, 512], F32, name="p_out", tag="p_out")
            for ec in range(4):
                src = outT_sb[:].rearrange("p (b ec) -> p ec b", ec=4)[:, ec, :]
                nc.tensor.transpose(p_out[0:nb, ec * 128:(ec + 1) * 128],
                                    src, ident32[:])
            out_sb = const.tile([nb, 512], F32, name="out_sb", tag="out_sb")
            nc.vector.tensor_copy(out_sb[0:nb, :], p_out[0:nb, :])
            nc.sync.dma_start(out_ap[:, :], out_sb[0:nb, :])


# ======== runner.py ========

NCORES = 8
_B_TOTAL = 256
_NB = _B_TOTAL // NCORES  # 32
_NBLK = 4

_built = {}


def _build(nb, nblk, has_bias):
    key = (nb, nblk, has_bias)
    if key in _built:
        return _built[key]
    import concourse.bacc as bacc
    import concourse.tile as tile

    nc = bacc.Bacc(trn_type="TRN2", target_bir_lowering=False, debug=False,
                   num_devices=NCORES)
    f16 = mybir.dt.float16
    f32 = mybir.dt.float32
    i8 = mybir.dt.int8
    not_ = nb * Nw // 128
    ntc = not_ // NWCHUNK
    nto = nb * No // 128
    ins = {
        "xoq": nc.dram_tensor("xoq", [nb * No, 512], i8, kind="ExternalInput").ap(),
        "xosc": nc.dram_tensor("xosc", [128, nto], f32, kind="ExternalInput").ap(),
        "wh": nc.dram_tensor("wh", [512, 1024], f16, kind="ExternalInput").ap(),
        **{f"xwq{i}": nc.dram_tensor(f"xwq{i}", [ntc * 128, 512], i8,
                                     kind="ExternalInput").ap()
           for i in range(NWCHUNK)},
        **{f"xwsc{i}": nc.dram_tensor(f"xwsc{i}", [128, ntc], f32,
                                      kind="ExternalInput").ap()
           for i in range(NWCHUNK)},
        "swp": nc.dram_tensor("swp", [128, nb * 8], f16, kind="ExternalInput").ap(),
        "sA2": nc.dram_tensor("sA2", [1, nb * 148], f16, kind="ExternalInput").ap(),
    }
    if has_bias:
        ins["bias128"] = nc.dram_tensor("bias128", [128, 4], f32,
                                        kind="ExternalInput").ap()
        ins["biasrow"] = nc.dram_tensor("biasrow", [1, 512], f16,
                                        kind="ExternalInput").ap()
    out_ap = nc.dram_tensor("out", [nb, 512], f32, kind="ExternalOutput").ap()
    with tile.TileContext(nc) as tc:
        build_gat(tc, out_ap, ins, nb=nb, nblk=nblk, has_bias=has_bias)
    nc.compile()
    _built[key] = nc
    return nc


# ---- host-side packing (jax cpu jit, multithreaded) ----

_prep_jit = None


def _get_prep_jit():
    global _prep_jit
    if _prep_jit is not None:
        return _prep_jit
    import functools
    import jax
    import jax.numpy as jnp

    cpu = jax.devices("cpu")[0]

    @functools.partial(jax.jit, static_argnums=(1,))
    def _prep_wchunk(word_embs, i):
        # quantize word rows of upload chunk i: per-core rows
        # [i*rows_pc, (i+1)*rows_pc) with rows_pc = nb*Nw/NWCHUNK
        B = word_embs.shape[0]
        nb = B // NCORES
        rows_pc = nb * Nw // NWCHUNK
        ntc = rows_pc // 128
        wf = word_embs.reshape(NCORES, NWCHUNK, rows_pc, D)[:, i]
        wf = wf.reshape(NCORES * rows_pc, D)
        # per-row scale = 4.4x RMS of a 128-col sample (clipped below): the
        # sum-of-squares reduce vectorizes far better than an amax pass here
        wam = jnp.maximum(
            jnp.sqrt(jnp.mean(wf[:, :128] * wf[:, :128], axis=1)) * 4.4, 1e-20)
        ws = wam / 127.0
        q = jnp.clip(jnp.rint(wf * (1.0 / ws)[:, None]), -127, 127).astype(jnp.int8)
        sc = ws.reshape(NCORES, ntc, 128).transpose(0, 2, 1).reshape(
            NCORES * 128, ntc).astype(jnp.float32)
        return q, sc

    @jax.jit
    def _prep_rest(word_embs, object_embs, W, att_src, att_dst):
        B = word_embs.shape[0]
        nb = B // NCORES
        nbl = nb // _NBLK
        wf = word_embs.reshape(B * Nw, D)
        of = object_embs.reshape(B * No, D)
        oam = jnp.maximum(
            jnp.sqrt(jnp.mean(of[:, :128] * of[:, :128], axis=1)) * 4.4, 1e-20)
        osc = oam / 127.0
        xoq = jnp.clip(jnp.rint(of * (1.0 / osc)[:, None]), -127, 127).astype(jnp.int8)
        nto = nb * No // 128
        xosc = osc.reshape(NCORES, nto, 128).transpose(0, 2, 1).reshape(
            NCORES * 128, nto).astype(jnp.float32)
        # exact attention scores
        Wr = W.reshape(D, H, E)
        wa_src = jnp.einsum('dhe,he->dh', Wr, att_src)
        wa_dst = jnp.einsum('dhe,he->dh', Wr, att_dst)
        waf = jnp.concatenate([wa_src, wa_dst], axis=1)  # [D, 4]
        s_w = wf @ waf   # [B*Nw, 4]
        s_o = of @ wa_src  # [B*No, 2]
        # swp [core*128, nb*8]: col = blk*nwt*4 + wt*4 + f ; row ~ (core, p)
        nwt = _NBLK * 2
        swp = s_w.reshape(NCORES, nbl, nwt, 128, 4).transpose(0, 3, 1, 2, 4)
        swp = swp.reshape(NCORES * 128, nb * 8).astype(jnp.float16)
        # sA2 [core, nb*148]: col = b*148 + whi*74 + h*37 + n
        so = s_o.reshape(NCORES, nb, No, H).transpose(0, 1, 3, 2)  # [c, b, h, n]
        sA2 = jnp.zeros((NCORES, nb, 2, H, 37), jnp.float16)
        sA2 = sA2.at[:, :, :, :, 0:No].set(
            so[:, :, None, :, :].astype(jnp.float16))
        sA2 = sA2.reshape(NCORES, nb * 148)
        # wh replicated
        wh = jnp.tile(W.astype(jnp.float16), (NCORES, 1))
        return xoq, xosc, wh, swp, sA2

    _prep_jit = (_prep_wchunk, _prep_rest, cpu)
    return _prep_jit


# ---- cached PJRT dispatch (one jit closure per build, reused warm) ----

_disp = {}


def _get_disp(nb, nblk, has_bias):
    key = (nb, nblk, has_bias)
    if key in _disp:
        return _disp[key]
    import jax
    from jax.sharding import Mesh, PartitionSpec, NamedSharding
    from jax.experimental.shard_map import shard_map
    from concourse import bass2jax

    nc = _build(nb, nblk, has_bias)
    bass2jax.install_neuronx_cc_hook()
    assert nc.dbg_addr is None or not nc.dbg_callbacks
    partition_name = nc.partition_id_tensor.name if nc.partition_id_tensor else None

    in_names, out_names, out_avals, out_shapes = [], [], [], []
    for alloc in nc.m.functions[0].allocations:
        if not isinstance(alloc, mybir.MemoryLocationSet):
            continue
        name = alloc.memorylocations[0].name
        if alloc.kind == "ExternalInput":
            if name != partition_name:
                in_names.append(name)
        elif alloc.kind == "ExternalOutput":
            shape = tuple(alloc.tensor_shape)
            dtype = mybir.dt.np(alloc.dtype)
            out_names.append(name)
            out_avals.append(jax.core.ShapedArray(shape, dtype))
            out_shapes.append((shape, dtype))
    n_params = len(in_names)
    n_outs = len(out_avals)
    in_names_all = list(in_names) + list(out_names)
    if partition_name is not None:
        in_names_all.append(partition_name)
    extra = []
    if nc.dbg_addr is not None:
        in_names_all.append(nc.dbg_addr.name)
        extra.append(np.zeros((1, 2), np.uint32))

    donate = tuple(range(n_params, n_params + n_outs))

    def _body(*args):
        operands = list(args)
        if partition_name is not None:
            operands.append(bass2jax.partition_id_tensor())
        outs = bass2jax._bass_exec_p.bind(
            *operands,
            out_avals=tuple(out_avals),
            in_names=tuple(in_names_all),
            out_names=tuple(out_names),
            lowering_input_output_aliases=(),
            sim_require_finite=True,
            sim_require_nnan=True,
            nc=nc,
        )
        return tuple(outs)

    devices = jax.devices()[:NCORES]
    mesh = Mesh(np.asarray(devices), ("core",))
    nargs = n_params + n_outs + len(extra)
    in_specs = (PartitionSpec("core"),) * nargs
    out_specs = (PartitionSpec("core"),) * n_outs
    sharded = jax.jit(
        shard_map(_body, mesh=mesh, in_specs=in_specs, out_specs=out_specs,
                  check_rep=False),
        donate_argnums=donate, keep_unused=True,
    )
    sh = NamedSharding(mesh, PartitionSpec("core"))
    d = {
        "sharded": sharded, "sharding": sh, "in_names": in_names,
        "out_shapes": out_shapes, "extra": extra,
    }
    _disp[key] = d
    return d


def _run(inputs, trace=False):
    import jax

    object_embs = np.asarray(inputs["object_embs"], np.float32)
    word_embs = np.asarray(inputs["word_embs"], np.float32)
    W = np.asarray(inputs["W"], np.float32)
    att_src = np.asarray(inputs["att_src"], np.float32)
    att_dst = np.asarray(inputs["att_dst"], np.float32)
    bias = np.asarray(inputs["bias"], np.float32)
    has_bias = bool(np.any(bias))
    B = object_embs.shape[0]
    nb = B // NCORES

    (prep_wchunk, prep_rest, cpu) = _get_prep_jit()
    if trace:
        # profiling path: per-core in_maps through run_bass_kernel_spmd
        with jax.default_device(cpu):
            chunks = [prep_wchunk(word_embs, i) for i in range(NWCHUNK)]
            rest = prep_rest(word_embs, object_embs, W, att_src, att_dst)
        host = {}
        for i, (q, sc) in enumerate(chunks):
            host[f"xwq{i}"] = np.asarray(q)
            host[f"xwsc{i}"] = np.asarray(sc)
        for k, v in zip(["xoq", "xosc", "wh", "swp", "sA2"], rest):
            host[k] = np.asarray(v)
        if has_bias:
            host["bias128"] = np.tile(
                np.ascontiguousarray(bias.reshape(4, 128).T.astype(np.float32)),
                (NCORES, 1))
            host["biasrow"] = np.tile(
                bias.reshape(1, 512).astype(np.float16), (NCORES, 1))
        from concourse import bass_utils
        nc = _build(nb, _NBLK, has_bias)
        in_maps = []
        for core in range(NCORES):
            m = {}
            for k, v in host.items():
                rows = v.shape[0] // NCORES
                m[k] = np.ascontiguousarray(v[core * rows:(core + 1) * rows])
            in_maps.append(m)
        res = bass_utils.run_bass_kernel_spmd(
            nc, in_maps, core_ids=list(range(NCORES)), trace=True)
        out = np.concatenate([r["out"] for r in res.results], axis=0)
        return out, res

    d = _get_disp(nb, _NBLK, has_bias)
    sh = d["sharding"]
    puts = {}
    with jax.default_device(cpu):
        # dispatch all host prep asynchronously (XLA-CPU queues them in order)
        chunks = [prep_wchunk(word_embs, i) for i in range(NWCHUNK)]
        rest = prep_rest(word_embs, object_embs, W, att_src, att_dst)
    # as each chunk's quantize completes, start its upload; the axon link
    # streams in the background while later chunks still compute
    for i, (q, sc) in enumerate(chunks):
        a, b = jax.device_put([np.asarray(q), np.asarray(sc)], [sh, sh])
        puts[f"xwq{i}"] = a
        puts[f"xwsc{i}"] = b
    ks = ["xoq", "xosc", "wh", "swp", "sA2"]
    vals = [np.asarray(v) for v in rest]
    puts.update(dict(zip(ks, jax.device_put(vals, [sh] * len(vals)))))
    if has_bias:
        puts["bias128"] = jax.device_put(np.tile(
            np.ascontiguousarray(bias.reshape(4, 128).T.astype(np.float32)),
            (NCORES, 1)), sh)
        puts["biasrow"] = jax.device_put(np.tile(
            bias.reshape(1, 512).astype(np.float16), (NCORES, 1)), sh)
    args = [puts[k] for k in d["in_names"]]
    # output operands are donated buffers; the kernel writes every element,
    # so on warm calls recycle the previous device-resident outputs instead
    # of uploading fresh zero buffers
    prev = d.get("prev_out")
    if prev is not None:
        outbufs = prev
    else:
        outbufs = [np.zeros((NCORES * s[0], *s[1:]), dt)
                   for (s, dt) in d["out_shapes"]]
    out_arrs = d["sharded"](*args, *outbufs, *d["extra"])
    out = np.asarray(out_arrs[0])
    d["prev_out"] = list(out_arrs)
    return out, None


def kernel(**inputs) -> np.ndarray:
    return _run(inputs, trace=False)[0]


# revision 21
# speedup vs baseline: 1.4010x; 1.1888x over previous
"""Trainium2 Bass kernel for nn_ObjectWordGAT (8-core data parallel).

Self-contained: accepts FULL inputs, shards batch across 8 NeuronCores,
returns the FULL [256, 512] fp32 output.

Warm-path design (the wall clock is dominated by the ~73MB/s axon link and
~0.1s RPC latencies, not HW exec):
  - embeddings are uploaded as per-row int8 (natural row-major layout), and
    dequantized + transposed on-chip (scalar engine affine + PE transposes),
    eliminating both the host-side transpose and half the upload bytes;
  - attention scores s = x @ (W @ att) are computed exactly on host (tiny
    sgemm) and uploaded pre-packed (~0.6MB), removing the quantization error
    from the logit path;
  - wm (head-mean projection) is derived on-chip from wh;
  - the jitted shard_map dispatch closure is built once and cached, so warm
    calls skip retracing/recompiling;
  - all device_puts are issued asynchronously and overlap each other.
"""
import numpy as np
import concourse.mybir as mybir


# ======== gat_core.py ========

from contextlib import ExitStack

from concourse.masks import make_identity

F16 = mybir.dt.float16
F32 = mybir.dt.float32
I8 = mybir.dt.int8
AF = mybir.ActivationFunctionType
ALU = mybir.AluOpType
AX = mybir.AxisListType

D = 512
H = 2
E = 512
No = 36
Nw = 256
NEG = 0.2
NWCHUNK = 4  # word rows are uploaded in this many pipelined chunks


def build_gat(tc, out_ap, ins, nb=32, nblk=4, has_bias=False, dbg=None):
    def tap(name, ap):
        if dbg is not None and name in dbg:
            tc.nc.sync.dma_start(dbg[name][:], ap)

    nc = tc.nc
    xoq, xosc = ins["xoq"], ins["xosc"]
    wh, swp, sA2 = ins["wh"], ins["swp"], ins["sA2"]
    RW, RO = nb * Nw, nb * 64  # obj rows padded to 64 per b
    ROP = nb * No              # packed obj rows (36 per b)
    nbl = nb // nblk
    not_ = nb * Nw // 128      # word row tiles per core
    ntc = not_ // NWCHUNK      # word row tiles per upload chunk
    nto = ROP // 128           # obj row tiles per core (1152/128 = 9)
    assert ROP % 128 == 0
    assert nb % nblk == 0 and nblk % 2 == 0

    ctx = ExitStack()
    with ctx:
        const = ctx.enter_context(tc.tile_pool(name="const", bufs=1))
        # ---- constants ----
        wh_sb = [const.tile([128, 1024], F16, name=f"wh{c}", tag=f"wh{c}") for c in range(4)]
        wm_sb = [const.tile([128, 512], F16, name=f"wm{c}", tag=f"wm{c}") for c in range(4)]
        for c in range(4):
            sl = slice(c * 128, (c + 1) * 128)
            nc.sync.dma_start(wh_sb[c][:], wh[sl, :])
        ident16 = const.tile([128, 128], F16, name="id16", tag="id16")
        ident32 = const.tile([128, 128], F32, name="id32", tag="id32")
        make_identity(nc, ident16[:])
        make_identity(nc, ident32[:])
        ones16 = const.tile([1, 128], F16, name="ones16", tag="ones16")
        nc.vector.memset(ones16[:], 1.0)
        # wm = 0.5 * (wh_head0 + wh_head1), on-chip
        for c in range(4):
            nc.vector.tensor_add(wm_sb[c][:], wh_sb[c][:, 0:512], wh_sb[c][:, 512:1024])
            nc.scalar.mul(wm_sb[c][:], wm_sb[c][:], 0.5)
        # packed attention-score constants (computed on host, exact)
        swp_sb = const.tile([128, nbl * nblk * 8], F16, name="swp", tag="swp")
        nc.sync.dma_start(swp_sb[:], swp[:, :])
        sA2_sb = const.tile([1, nb * 148], F16, name="sA2", tag="sA2")
        nc.sync.dma_start(sA2_sb[:], sA2[:, :])
        # quant scales
        xwsc_sb = [const.tile([128, ntc], F32, name=f"xwsc{i}", tag=f"xwsc{i}")
                   for i in range(NWCHUNK)]
        for i in range(NWCHUNK):
            nc.sync.dma_start(xwsc_sb[i][:], ins[f"xwsc{i}"][:, :])
        xosc_sb = const.tile([128, nto], F32, name="xosc", tag="xosc")
        nc.sync.dma_start(xosc_sb[:], xosc[:, :])
        if has_bias:
            bias_sb = const.tile([128, 4], F32, name="bias128", tag="bias128")
            nc.sync.dma_start(bias_sb[:], ins["bias128"][:, :])
            biasrow_sb = const.tile([1, 512], F16, name="biasrow", tag="biasrow")
            nc.sync.dma_start(biasrow_sb[:], ins["biasrow"][:, :])

        # resident results
        ngrp2 = nb // 2  # obj rows padded: 2 b per 128-row tile
        xto_sb = [const.tile([128, RO], F16, name=f"xto{c}", tag=f"xto{c}") for c in range(4)]
        hobj_sb = const.tile([128, ngrp2 * 1024], F16, name="hobj", tag="hobj")
        uoT_sb = const.tile([128, 4 * RO], F16, name="uoT", tag="uoT")
        outT_sb = const.tile([128, nb * 4], F32, name="outT", tag="outT")

        # ================= PHASE 0: objects dequant + transpose =================
        # xoq [ROP, 512] int8 natural -> xto_sb[c] [128, RO] f16 (64-padded per b)
        with (
            tc.tile_pool(name="sb0", bufs=2) as sb0,
            tc.tile_pool(name="ps0", bufs=2, space="PSUM") as ps0,
        ):
            xtoP_sb = [const.tile([128, ROP], F16, name=f"xtoP{c}", tag=f"xtoP{c}")
                       for c in range(4)]
            for t in range(nto):
                xq_t = sb0.tile([128, 512], I8, name="xq", tag="xq")
                nc.sync.dma_start(xq_t[:], xoq[t * 128:(t + 1) * 128, :])
                xf_t = sb0.tile([128, 512], F16, name="xf", tag="xf")
                nc.scalar.mul(xf_t[:], xq_t[:], xosc_sb[:, t:t + 1])
                pt = ps0.tile([128, 512], F16, name="pt", tag="pt")
                for c in range(4):
                    nc.tensor.transpose(pt[:, c * 128:(c + 1) * 128],
                                        xf_t[:, c * 128:(c + 1) * 128], ident16[:])
                for c in range(4):
                    eng = nc.scalar.copy if (t + c) % 2 == 0 else nc.vector.tensor_copy
                    eng(xtoP_sb[c][:, t * 128:(t + 1) * 128],
                        pt[:, c * 128:(c + 1) * 128])
            # pad 36 -> 64 per b
            for c in range(4):
                nc.gpsimd.memset(
                    xto_sb[c][:].rearrange("p (b n) -> p b n", n=64)[:, :, No:64], 0.0)
                nc.vector.tensor_copy(
                    xto_sb[c][:].rearrange("p (b n) -> p b n", n=64)[:, :, 0:No],
                    xtoP_sb[c][:].rearrange("p (b n) -> p b n", n=No))
        tap("xto0", xto_sb[0][:])

        # ================= PHASE A: objects =================
        with tc.tile_pool(name="psA", bufs=2, space="PSUM") as psA:
            for g in range(ngrp2):
                pt = psA.tile([128, 1024], F32, name="phobj", tag="phobj")
                for he in range(2):
                    for c in range(4):
                        nc.tensor.matmul(
                            pt[:, he * 512:(he + 1) * 512],
                            lhsT=xto_sb[c][:, 128 * g:128 * (g + 1)],
                            rhs=wh_sb[c][:, he * 512:(he + 1) * 512],
                            start=(c == 0), stop=(c == 3),
                        )
                eng = nc.scalar.copy if g % 2 == 0 else nc.vector.tensor_copy
                eng(hobj_sb[:, g * 1024:(g + 1) * 1024], pt[:, :])

        with tc.tile_pool(name="psB", bufs=2, space="PSUM") as psB:
            # upd_obj^T = Wm.T @ Xo^T (+bias on evac)
            nchunks = [(i, min(512, RO - i)) for i in range(0, RO, 512)]
            for ec in range(4):
                for n0, nn in nchunks:
                    pt = psB.tile([128, 512], F32, name="puoT", tag="puoT")
                    for c in range(4):
                        nc.tensor.matmul(
                            pt[:, 0:nn],
                            lhsT=wm_sb[c][:, ec * 128:(ec + 1) * 128],
                            rhs=xto_sb[c][:, n0:n0 + nn],
                            start=(c == 0), stop=(c == 3),
                        )
                    dst = uoT_sb[:, ec * RO + n0: ec * RO + n0 + nn]
                    if has_bias:
                        nc.scalar.activation(dst, pt[:, 0:nn], AF.Identity,
                                             bias=bias_sb[:, ec:ec + 1])
                    elif (ec * len(nchunks) + n0 // 512) % 2 == 0:
                        nc.scalar.copy(dst, pt[:, 0:nn])
                    else:
                        nc.vector.tensor_copy(dst, pt[:, 0:nn])

        tap("hobj", hobj_sb[:])
        tap("uoT", uoT_sb[:])
        tap("sA2", sA2_sb[:])

        # ================= PHASE B: word blocks =================
        with (
            tc.tile_pool(name="sbB", bufs=2) as sbB,
            tc.tile_pool(name="ps_hw", bufs=2, space="PSUM") as ps_hw,
            tc.tile_pool(name="ps_mid", bufs=2, space="PSUM") as ps_mid,
            tc.tile_pool(name="ps_sm", bufs=2, space="PSUM") as ps_sm,
            tc.tile_pool(name="ps_aT", bufs=1, space="PSUM") as ps_aT,
        ):
            for blk in range(nbl):
                gw0 = blk * nblk * Nw  # first word row of block
                nwt = nblk * 2  # 128-row word tiles in block
                ng = nblk * 4   # (bi, whi, h) groups in block
                # ---- dequant + transpose words of this block ----
                xtw_sb = [sbB.tile([128, nblk * 256], F16, name=f"xtw{c}", tag=f"xtw{c}")
                          for c in range(4)]
                for wt8 in range(nwt):
                    t = blk * nwt + wt8
                    ch, tci = t // ntc, t % ntc
                    xq_t = sbB.tile([128, 512], I8, name="xqw", tag="xqw")
                    nc.sync.dma_start(
                        xq_t[:], ins[f"xwq{ch}"][tci * 128:(tci + 1) * 128, :])
                    xf_t = sbB.tile([128, 512], F16, name="xfw", tag="xfw")
                    nc.scalar.mul(xf_t[:], xq_t[:], xwsc_sb[ch][:, tci:tci + 1])
                    pt = ps_aT.tile([128, 512], F16, name="paT", tag="paT")
                    for c in range(4):
                        nc.tensor.transpose(pt[:, c * 128:(c + 1) * 128],
                                            xf_t[:, c * 128:(c + 1) * 128], ident16[:])
                    for c in range(4):
                        eng = nc.scalar.copy if (wt8 + c) % 2 == 0 else nc.vector.tensor_copy
                        eng(xtw_sb[c][:, wt8 * 128:(wt8 + 1) * 128],
                            pt[:, c * 128:(c + 1) * 128])

                # ---- s_word: packed slice of the host-computed scores ----
                sw_sb = swp_sb[:, blk * nwt * 4:(blk + 1) * nwt * 4]
                if blk == 0:
                    tap("sw", sw_sb)

                # ---- spread [128, nblk*148] = s_dst col per (bi,whi,h) ----
                spread_sb = sbB.tile([128, nblk * 148], F16, name="spread", tag="spread")
                src = sw_sb.rearrange("p (b whi f) -> p b whi f",
                                      b=nblk, whi=2)[:, :, :, 2:4]
                src = src.broadcast_to([128, nblk, 2, 2, 37])
                dst = spread_sb[:].rearrange("p (b whi h n) -> p b whi h n",
                                             b=nblk, whi=2, h=2)
                nc.vector.tensor_copy(dst, src)
                # self column (n=36): s_src + s_dst
                swg = sw_sb.rearrange("p (b whi f) -> p b whi f",
                                      b=nblk, whi=2)
                nc.vector.tensor_add(
                    dst[:, :, :, :, 36:37].rearrange("p b whi h n -> p b whi (h n)"),
                    dst[:, :, :, :, 36:37].rearrange("p b whi h n -> p b whi (h n)"),
                    swg[:, :, :, 0:2])

                # ---- L psums + lrelu + exp ----
                L2_sb = sbB.tile([128, nblk * 148], F32, name="L2", tag="L2")
                half = nblk * 148 // 2
                for hf in range(2):
                    p_L = ps_sm.tile([128, half], F32, name="sm", tag="sm")
                    nc.tensor.matmul(
                        p_L[:], lhsT=ones16[:],
                        rhs=sA2_sb[0:1, blk * nblk * 148 + hf * half:][:, 0:half],
                        start=True, stop=False)
                    nc.tensor.matmul(
                        p_L[:], lhsT=ident16[:],
                        rhs=spread_sb[:, hf * half:(hf + 1) * half],
                        start=False, stop=True)
                    ltmp = sbB.tile([128, half], F16, name="ltmp", tag="ltmp")
                    nc.scalar.mul(ltmp[:], p_L[:], NEG)
                    nc.vector.tensor_max(
                        L2_sb[:, hf * half:(hf + 1) * half], p_L[:], ltmp[:])
                expL_sb = sbB.tile([128, nblk * 148], F32, name="expL", tag="expL")
                nc.scalar.activation(expL_sb[:], L2_sb[:], AF.Exp)
                if blk == 0:
                    tap("L2", L2_sb[:])
                    tap("expL", expL_sb[:])

                # ---- den, r, alpha, c ----
                expg = expL_sb[:].rearrange("p (g n) -> p g n", n=37)
                den_sb = sbB.tile([128, ng], F32, name="den", tag="den")
                nc.vector.tensor_reduce(den_sb[:], expg, axis=AX.X, op=ALU.add)
                r_sb = sbB.tile([128, ng], F32, name="r", tag="r")
                nc.vector.reciprocal(r_sb[:], den_sb[:])
                nc.vector.tensor_scalar_mul(r_sb[:], r_sb[:], 0.5)
                alpha_sb = sbB.tile([128, ng * 64], F16, name="alpha", tag="alpha")
                nc.gpsimd.memset(
                    alpha_sb[:].rearrange("p (g n) -> p g n", n=64)[:, :, 36:64],
                    0.0)
                rbc = r_sb[:].broadcast_to([128, ng, 36])
                nc.vector.tensor_mul(
                    alpha_sb[:].rearrange("p (g n) -> p g n", n=64)[:, :, 0:36],
                    expg[:, :, 0:36], rbc)
                c_sb = sbB.tile([128, ng], F32, name="c", tag="c")
                nc.vector.tensor_mul(
                    c_sb[:],
                    expg[:, :, 36:37].rearrange("p g n -> p (g n)"), r_sb[:])
                if blk == 0:
                    tap("den", den_sb[:])
                    tap("alpha", alpha_sb[:])
                    tap("c", c_sb[:])

                # ---- alpha transposes -> aT [128, (nblk/2)*512] ----
                # partition half = b parity; col = pair*512 + h*256 + whi*128
                aT_sb = sbB.tile([128, (nblk // 2) * 512], F16, name="aT",
                                 tag="aT")
                for pr in range(nblk // 2):
                    p_aTt = ps_aT.tile([128, 512], F16, name="paT", tag="paT")
                    for pb in range(2):
                        bi = pr * 2 + pb
                        for whi in range(2):
                            for h in range(2):
                                g = (bi * 2 + whi) * 2 + h
                                nc.tensor.transpose(
                                    p_aTt[64 * pb:64 * pb + 64,
                                          h * 256 + whi * 128:][:, 0:128],
                                    alpha_sb[:, g * 64:(g + 1) * 64],
                                    ident16[:],
                                    tile_position=(0, 64 * pb),
                                )
                    nc.vector.tensor_copy(aT_sb[:, pr * 512:(pr + 1) * 512],
                                          p_aTt[:])

                if blk == 0:
                    tap("aT", aT_sb[:])
                # ---- h_word proj + t + msg + uw per (bi, whi) ----
                t_sb = sbB.tile([128, nwt * 512], F16, name="t", tag="t")
                uw_sb = sbB.tile([128, nwt * 512], F16, name="uw", tag="uw")
                for bi in range(nblk):
                    b = blk * nblk + bi
                    for whi in range(2):
                        wt = bi * 2 + whi
                        g = wt * 2  # (bi, whi, h=0)
                        p_he0 = ps_hw.tile([128, 512], F32, name="hw", tag="hw")
                        for c in range(4):
                            nc.tensor.matmul(
                                p_he0[:],
                                lhsT=xtw_sb[c][:, wt * 128:(wt + 1) * 128],
                                rhs=wh_sb[c][:, 0:512],
                                start=(c == 0), stop=(c == 3))
                        t0_sb = sbB.tile([128, 512], F16, name="t0", tag="t0")
                        nc.scalar.mul(t0_sb[:], p_he0[:], c_sb[:, g:g + 1])
                        p_he1 = ps_hw.tile([128, 512], F32, name="hw", tag="hw")
                        for c in range(4):
                            nc.tensor.matmul(
                                p_he1[:],
                                lhsT=xtw_sb[c][:, wt * 128:(wt + 1) * 128],
                                rhs=wh_sb[c][:, 512:1024],
                                start=(c == 0), stop=(c == 3))
                        t1_sb = sbB.tile([128, 512], F16, name="t1", tag="t1")
                        nc.vector.tensor_scalar_mul(t1_sb[:], p_he1[:],
                                                    c_sb[:, g + 1:g + 2])
                        nc.gpsimd.tensor_add(t_sb[:, wt * 512:(wt + 1) * 512],
                                             t0_sb[:], t1_sb[:])

                        # msg: two K=36 matmuls at row base 64*(b%2)
                        p_msg = ps_mid.tile([128, 512], F32, name="mid", tag="mid")
                        gq, go = b // 2, 64 * (b % 2)
                        acol = (bi // 2) * 512 + whi * 128
                        nc.tensor.matmul(
                            p_msg[:],
                            lhsT=aT_sb[go:go + 36, acol:acol + 128],
                            rhs=hobj_sb[go:go + 36, gq * 1024:gq * 1024 + 512],
                            start=True, stop=False,
                            tile_position=(go, 0))
                        nc.tensor.matmul(
                            p_msg[:],
                            lhsT=aT_sb[go:go + 36, acol + 256:acol + 256 + 128],
                            rhs=hobj_sb[go:go + 36,
                                        gq * 1024 + 512:gq * 1024 + 1024],
                            start=False, stop=not has_bias,
                            tile_position=(go, 0))
                        if has_bias:
                            nc.tensor.matmul(p_msg[:], lhsT=ones16[:],
                                             rhs=biasrow_sb[:],
                                             start=False, stop=True)
                        nc.vector.tensor_add(
                            uw_sb[:, wt * 512:(wt + 1) * 512], p_msg[:],
                            t_sb[:, wt * 512:(wt + 1) * 512])

                if blk == 0:
                    tap("t", t_sb[:])
                    tap("uw", uw_sb[:])
                # ---- uw transposes -> uwT [128, nblk*4*256] ----
                uwT_sb = sbB.tile([128, nblk * 4 * 256], F16, name="uwT", tag="uwT")
                for bi in range(nblk):
                    for ec in range(4):
                        p_uwT = ps_mid.tile([128, 256], F16, name="mid", tag="mid")
                        for whi in range(2):
                            nc.tensor.transpose(
                                p_uwT[:, whi * 128:(whi + 1) * 128],
                                uw_sb[:, (bi * 2 + whi) * 512 + ec * 128:][:, 0:128],
                                ident16[:])
                        dst = uwT_sb[:, (bi * 4 + ec) * 256:
                                     (bi * 4 + ec + 1) * 256]
                        if ec % 2 == 0:
                            nc.scalar.copy(dst, p_uwT[:])
                        else:
                            nc.vector.tensor_copy(dst, p_uwT[:])

                if blk == 0:
                    tap("uwT", uwT_sb[:])
                # ---- C + softmax + attnT ----
                p_attnT = ps_aT.tile([128, nblk * 2 * 36], F16, name="pattnT", tag="pattnT")
                for pair in range(nblk // 2):
                    p_C = ps_sm.tile([128, 256], F32, name="sm", tag="sm")
                    for pb in range(2):
                        bi = pair * 2 + pb
                        b = blk * nblk + bi
                        for ec in range(4):
                            nc.tensor.matmul(
                                p_C[64 * pb:64 * pb + 36, :],
                                lhsT=uoT_sb[:, ec * RO + b * 64:
                                            ec * RO + b * 64 + 36],
                                rhs=uwT_sb[:, (bi * 4 + ec) * 256:
                                           (bi * 4 + ec + 1) * 256],
                                start=(ec == 0), stop=(ec == 3),
                                tile_position=(0, 64 * pb))
                    negmax = sbB.tile([128, 1], F32, name="negmax", tag="negmax")
                    expC = sbB.tile([128, 256], F16, name="expC", tag="expC")
                    den2 = sbB.tile([128, 1], F32, name="den2", tag="den2")
                    rden = sbB.tile([128, 1], F32, name="rden", tag="rden")
                    attn = sbB.tile([128, 256], F16, name="attn", tag="attn")
                    for pb in range(2):
                        rs = slice(64 * pb, 64 * pb + 36)
                        nc.vector.tensor_reduce(negmax[rs], p_C[rs, :], axis=AX.X,
                                                op=ALU.max, negate=True)
                        nc.scalar.activation(expC[rs, :], p_C[rs, :], AF.Exp,
                                             bias=negmax[rs], accum_out=den2[rs])
                        nc.vector.reciprocal(rden[rs], den2[rs])
                        nc.vector.tensor_scalar_mul(rden[rs], rden[rs],
                                                    1.0 / 36.0)
                        nc.vector.tensor_scalar_mul(attn[rs, :], expC[rs, :],
                                                    rden[rs])
                    if blk == 0 and pair == 0:
                        tap("attn", attn[:])
                        tap("expC", expC[:])
                    for pb in range(2):
                        bi = pair * 2 + pb
                        for whi in range(2):
                            nc.tensor.transpose(
                                p_attnT[:, (bi * 2 + whi) * 36:
                                        (bi * 2 + whi + 1) * 36],
                                attn[64 * pb:64 * pb + 36,
                                     whi * 128:(whi + 1) * 128],
                                ident16[64 * pb:64 * pb + 36,
                                        64 * pb:64 * pb + 36],
                                tile_position=(64 * pb, 0))
                attnT_sb = sbB.tile([128, nblk * 2 * 36], F16, name="attnT", tag="attnT")
                nc.vector.tensor_copy(attnT_sb[:], p_attnT[:])
                if blk == 0:
                    tap("attnT", attnT_sb[:])

                # ---- weighted^T + final reduce ----
                for bi in range(nblk):
                    b = blk * nblk + bi
                    p_w = ps_sm.tile([128, 144], F32, name="sm", tag="sm")
                    for ec in range(4):
                        for whi in range(2):
                            nc.tensor.matmul(
                                p_w[:, ec * 36:(ec + 1) * 36],
                                lhsT=uw_sb[:, (bi * 2 + whi) * 512 +
                                           ec * 128:][:, 0:128],
                                rhs=attnT_sb[:, (bi * 2 + whi) * 36:
                                             (bi * 2 + whi + 1) * 36],
                                start=(whi == 0), stop=(whi == 1))
                    nc.vector.tensor_reduce(
                        outT_sb[:, b * 4:(b + 1) * 4],
                        p_w[:].rearrange("p (ec n) -> p ec n", n=36),
                        axis=AX.X, op=ALU.add)

        tap("outT", outT_sb[:])
        # ================= PHASE C: final transpose + store =================
        with tc.tile_pool(name="psC", bufs=1, space="PSUM") as psC:
            assert nb <= 128
            p_out = psC.tile([nb, 512], F32, name="p_out", tag="p_out")
            for ec in range(4):
                src = outT_sb[:].rearrange("p (b ec) -> p ec b", ec=4)[:, ec, :]
                nc.tensor.transpose(p_out[0:nb, ec * 128:(ec + 1) * 128],
                                    src, ident32[:])
            out_sb = const.tile([nb, 512], F32, name="out_sb", tag="out_sb")
            nc.vector.tensor_copy(out_sb[0:nb, :], p_out[0:nb, :])
            nc.sync.dma_start(out_ap[:, :], out_sb[0:nb, :])


# ======== runner.py ========

NCORES = 8
_B_TOTAL = 256
_NB = _B_TOTAL // NCORES  # 32
_NBLK = 4

_built = {}


def _build(nb, nblk, has_bias):
    key = (nb, nblk, has_bias)
    if key in _built:
        return _built[key]
    import concourse.bacc as bacc
    import concourse.tile as tile

    nc = bacc.Bacc(trn_type="TRN2", target_bir_lowering=False, debug=False,
                   num_devices=NCORES)
    f16 = mybir.dt.float16
    f32 = mybir.dt.float32
    i8 = mybir.dt.int8
    not_ = nb * Nw // 128
    ntc = not_ // NWCHUNK
    nto = nb * No // 128
    ins = {
        "xoq": nc.dram_tensor("xoq", [nb * No, 512], i8, kind="ExternalInput").ap(),
        "xosc": nc.dram_tensor("xosc", [128, nto], f32, kind="ExternalInput").ap(),
        "wh": nc.dram_tensor("wh", [512, 1024], f16, kind="ExternalInput").ap(),
        **{f"xwq{i}": nc.dram_tensor(f"xwq{i}", [ntc * 128, 512], i8,
                                     kind="ExternalInput").ap()
           for i in range(NWCHUNK)},
        **{f"xwsc{i}": nc.dram_tensor(f"xwsc{i}", [128, ntc], f32,
                                      kind="ExternalInput").ap()
           for i in range(NWCHUNK)},
        "swp": nc.dram_tensor("swp", [128, nb * 8], f16, kind="ExternalInput").ap(),
        "sA2": nc.dram_tensor("sA2", [1, nb * 148], f16, kind="ExternalInput").ap(),
    }
    if has_bias:
        ins["bias128"] = nc.dram_tensor("bias128", [128, 4], f32,
                                        kind="ExternalInput").ap()
        ins["biasrow"] = nc.dram_tensor("biasrow", [1, 512], f16,
                                        kind="ExternalInput").ap()
    out_ap = nc.dram_tensor("out", [nb, 512], f32, kind="ExternalOutput").ap()
    with tile.TileContext(nc) as tc:
        build_gat(tc, out_ap, ins, nb=nb, nblk=nblk, has_bias=has_bias)
    nc.compile()
    _built[key] = nc
    return nc


# ---- host-side packing (jax cpu jit, multithreaded) ----

_prep_jit = None


def _get_prep_jit():
    global _prep_jit
    if _prep_jit is not None:
        return _prep_jit
    import functools
    import jax
    import jax.numpy as jnp

    cpu = jax.devices("cpu")[0]

    @functools.partial(jax.jit, static_argnums=(1,))
    def _prep_wchunk(word_embs, i):
        # quantize word rows of upload chunk i: per-core rows
        # [i*rows_pc, (i+1)*rows_pc) with rows_pc = nb*Nw/NWCHUNK
        B = word_embs.shape[0]
        nb = B // NCORES
        rows_pc = nb * Nw // NWCHUNK
        ntc = rows_pc // 128
        wf = word_embs.reshape(NCORES, NWCHUNK, rows_pc, D)[:, i]
        wf = wf.reshape(NCORES * rows_pc, D)
        # per-row scale = 4.4x RMS of a 128-col sample (clipped below): the
        # sum-of-squares reduce vectorizes far better than an amax pass here
        wam = jnp.maximum(
            jnp.sqrt(jnp.mean(wf[:, :128] * wf[:, :128], axis=1)) * 4.4, 1e-20)
        ws = wam / 127.0
        q = jnp.clip(jnp.rint(wf * (1.0 / ws)[:, None]), -127, 127).astype(jnp.int8)
        sc = ws.reshape(NCORES, ntc, 128).transpose(0, 2, 1).reshape(
            NCORES * 128, ntc).astype(jnp.float32)
        return q, sc

    @jax.jit
    def _prep_rest(word_embs, object_embs, W, att_src, att_dst):
        B = word_embs.shape[0]
        nb = B // NCORES
        nbl = nb // _NBLK
        wf = word_embs.reshape(B * Nw, D)
        of = object_embs.reshape(B * No, D)
        oam = jnp.maximum(
            jnp.sqrt(jnp.mean(of[:, :128] * of[:, :128], axis=1)) * 4.4, 1e-20)
        osc = oam / 127.0
        xoq = jnp.clip(jnp.rint(of * (1.0 / osc)[:, None]), -127, 127).astype(jnp.int8)
        nto = nb * No // 128
        xosc = osc.reshape(NCORES, nto, 128).transpose(0, 2, 1).reshape(
            NCORES * 128, nto).astype(jnp.float32)
        # exact attention scores
        Wr = W.reshape(D, H, E)
        wa_src = jnp.einsum('dhe,he->dh', Wr, att_src)
        wa_dst = jnp.einsum('dhe,he->dh', Wr, att_dst)
        waf = jnp.concatenate([wa_src, wa_dst], axis=1)  # [D, 4]
        s_w = wf @ waf   # [B*Nw, 4]
        s_o = of @ wa_src  # [B*No, 2]
        # swp [core*128, nb*8]: col = blk*nwt*4 + wt*4 + f ; row ~ (core, p)
        nwt = _NBLK * 2
        swp = s_w.reshape(NCORES, nbl, nwt, 128, 4).transpose(0, 3, 1, 2, 4)
        swp = swp.reshape(NCORES * 128, nb * 8).astype(jnp.float16)
        # sA2 [core, nb*148]: col = b*148 + whi*74 + h*37 + n
        so = s_o.reshape(NCORES, nb, No, H).transpose(0, 1, 3, 2)  # [c, b, h, n]
        sA2 = jnp.zeros((NCORES, nb, 2, H, 37), jnp.float16)
        sA2 = sA2.at[:, :, :, :, 0:No].set(
            so[:, :, None, :, :].astype(jnp.float16))
        sA2 = sA2.reshape(NCORES, nb * 148)
        return xoq, xosc, swp, sA2

    _prep_jit = (_prep_wchunk, _prep_rest, cpu)
    return _prep_jit


# ---- cached PJRT dispatch (one jit closure per build, reused warm) ----

_disp = {}


def _get_disp(nb, nblk, has_bias):
    key = (nb, nblk, has_bias)
    if key in _disp:
        return _disp[key]
    import jax
    from jax.sharding import Mesh, PartitionSpec, NamedSharding
    from jax.experimental.shard_map import shard_map
    from concourse import bass2jax

    nc = _build(nb, nblk, has_bias)
    bass2jax.install_neuronx_cc_hook()
    assert nc.dbg_addr is None or not nc.dbg_callbacks
    partition_name = nc.partition_id_tensor.name if nc.partition_id_tensor else None

    in_names, out_names, out_avals, out_shapes = [], [], [], []
    for alloc in nc.m.functions[0].allocations:
        if not isinstance(alloc, mybir.MemoryLocationSet):
            continue
        name = alloc.memorylocations[0].name
        if alloc.kind == "ExternalInput":
            if name != partition_name:
                in_names.append(name)
        elif alloc.kind == "ExternalOutput":
            shape = tuple(alloc.tensor_shape)
            dtype = mybir.dt.np(alloc.dtype)
            out_names.append(name)
            out_avals.append(jax.core.ShapedArray(shape, dtype))
            out_shapes.append((shape, dtype))
    n_params = len(in_names)
    n_outs = len(out_avals)
    in_names_all = list(in_names) + list(out_names)
    if partition_name is not None:
        in_names_all.append(partition_name)
    extra = []
    if nc.dbg_addr is not None:
        in_names_all.append(nc.dbg_addr.name)
        extra.append(np.zeros((1, 2), np.uint32))

    donate = tuple(range(n_params, n_params + n_outs))

    def _body(*args):
        operands = list(args)
        if partition_name is not None:
            operands.append(bass2jax.partition_id_tensor())
        outs = bass2jax._bass_exec_p.bind(
            *operands,
            out_avals=tuple(out_avals),
            in_names=tuple(in_names_all),
            out_names=tuple(out_names),
            lowering_input_output_aliases=(),
            sim_require_finite=True,
            sim_require_nnan=True,
            nc=nc,
        )
        return tuple(outs)

    devices = jax.devices()[:NCORES]
    mesh = Mesh(np.asarray(devices), ("core",))
    nargs = n_params + n_outs + len(extra)
    in_specs = (PartitionSpec("core"),) * nargs
    out_specs = (PartitionSpec("core"),) * n_outs
    sharded = jax.jit(
        shard_map(_body, mesh=mesh, in_specs=in_specs, out_specs=out_specs,
                  check_rep=False),
        donate_argnums=donate, keep_unused=True,
    )
    sh = NamedSharding(mesh, PartitionSpec("core"))
    d = {
        "sharded": sharded, "sharding": sh, "in_names": in_names,
        "out_shapes": out_shapes, "extra": extra,
    }
    _disp[key] = d
    return d


def _run(inputs, trace=False):
    import jax

    object_embs = np.asarray(inputs["object_embs"], np.float32)
    word_embs = np.asarray(inputs["word_embs"], np.float32)
    W = np.asarray(inputs["W"], np.float32)
    att_src = np.asarray(inputs["att_src"], np.float32)
    att_dst = np.asarray(inputs["att_dst"], np.float32)
    bias = np.asarray(inputs["bias"], np.float32)
    has_bias = bool(np.any(bias))
    B = object_embs.shape[0]
    nb = B // NCORES

    (prep_wchunk, prep_rest, cpu) = _get_prep_jit()
    if trace:
        # profiling path: per-core in_maps through run_bass_kernel_spmd
        with jax.default_device(cpu):
            chunks = [prep_wchunk(word_embs, i) for i in range(NWCHUNK)]
            rest = prep_rest(word_embs, object_embs, W, att_src, att_dst)
        host = {}
        for i, (q, sc) in enumerate(chunks):
            host[f"xwq{i}"] = np.asarray(q)
            host[f"xwsc{i}"] = np.asarray(sc)
        for k, v in zip(["xoq", "xosc", "swp", "sA2"], rest):
            host[k] = np.asarray(v)
        host["wh"] = np.tile(np.ascontiguousarray(W.astype(np.float16)),
                             (NCORES, 1))
        if has_bias:
            host["bias128"] = np.tile(
                np.ascontiguousarray(bias.reshape(4, 128).T.astype(np.float32)),
                (NCORES, 1))
            host["biasrow"] = np.tile(
                bias.reshape(1, 512).astype(np.float16), (NCORES, 1))
        from concourse import bass_utils
        nc = _build(nb, _NBLK, has_bias)
        in_maps = []
        for core in range(NCORES):
            m = {}
            for k, v in host.items():
                rows = v.shape[0] // NCORES
                m[k] = np.ascontiguousarray(v[core * rows:(core + 1) * rows])
            in_maps.append(m)
        res = bass_utils.run_bass_kernel_spmd(
            nc, in_maps, core_ids=list(range(NCORES)), trace=True)
        out = np.concatenate([r["out"] for r in res.results], axis=0)
        return out, res

    d = _get_disp(nb, _NBLK, has_bias)
    sh = d["sharding"]
    puts = {}
    with jax.default_device(cpu):
        # dispatch all host prep asynchronously (XLA-CPU queues them in order)
        chunks = [prep_wchunk(word_embs, i) for i in range(NWCHUNK)]
        rest = prep_rest(word_embs, object_embs, W, att_src, att_dst)
    # as each chunk's quantize completes, start its upload; the axon link
    # streams in the background while later chunks still compute
    for i, (q, sc) in enumerate(chunks):
        a, b = jax.device_put([np.asarray(q), np.asarray(sc)], [sh, sh])
        puts[f"xwq{i}"] = a
        puts[f"xwsc{i}"] = b
    # wh depends only on W (a weight, not an activation): keep the uploaded
    # replicas device-resident across calls, re-uploading iff W changed
    wc = d.get("wcache")
    if wc is not None and np.array_equal(wc["W"], W):
        puts["wh"] = wc["wh_dev"]
    else:
        wh_np = np.tile(np.ascontiguousarray(W.astype(np.float16)), (NCORES, 1))
        puts["wh"] = jax.device_put(wh_np, sh)
        d["wcache"] = {"W": W.copy(), "wh_dev": puts["wh"]}
    ks = ["xoq", "xosc", "swp", "sA2"]
    vals = [np.asarray(v) for v in rest]
    puts.update(dict(zip(ks, jax.device_put(vals, [sh] * len(vals)))))
    if has_bias:
        puts["bias128"] = jax.device_put(np.tile(
            np.ascontiguousarray(bias.reshape(4, 128).T.astype(np.float32)),
            (NCORES, 1)), sh)
        puts["biasrow"] = jax.device_put(np.tile(
            bias.reshape(1, 512).astype(np.float16), (NCORES, 1)), sh)
    args = [puts[k] for k in d["in_names"]]
    # output operands are donated buffers; the kernel writes every element,
    # so on warm calls recycle the previous device-resident outputs instead
    # of uploading fresh zero buffers
    prev = d.get("prev_out")
    if prev is not None:
        outbufs = prev
    else:
        outbufs = [np.zeros((NCORES * s[0], *s[1:]), dt)
                   for (s, dt) in d["out_shapes"]]
    out_arrs = d["sharded"](*args, *outbufs, *d["extra"])
    out = np.asarray(out_arrs[0])
    d["prev_out"] = list(out_arrs)
    return out, None


def kernel(**inputs) -> np.ndarray:
    return _run(inputs, trace=False)[0]
